# revision 10
# baseline (speedup 1.0000x reference)
"""Causal self-attention (B=4, T=2048, D=1024, H=16, d=64) on 8 TRN2 cores.

Sharding: 8 cores = 4 batches x 2 head-groups (8 heads each). Each core
computes, for its (batch, head-group):
  qk^T = (x @ w_qk)^T           [1024, T]   (q^T rows 0..511, k^T rows 512..1023)
  v    = x @ w_v                [T, 512]    (+ ones column per head -> [.., 65])
  S^T  = K^T.T @ Q^T per head   [k, q] tiles, exp via ACT (scale 1/8 folded)
  P^T causal-masked, AV: oT[65, q] accumulates V'.T @ P^T  (row 64 = denom)
  attn^T = oT[0:64] * (1/denom) (denominator broadcast via K=1 matmul)
  out^T += w_proj_slice.T-chunks @ attn^T  -> [1024, T] partial
Host sums the two head-group partials per batch, adds biases' linear terms,
and transposes back. All matmuls run as float32r (full-rate fp32).
"""

import json

import numpy as np

B = 4
T = 2048
C = 1024
NH = 8          # heads per core
D = 64
TCH = 512       # q/time chunk
NKT = T // 128  # 16 k-tiles
NCH = T // TCH  # 4 chunks

_RUNNER = None


# ---------------------------------------------------------------- BIR legalize
def _legalize_bir_json(bir_bytes):
    """Stock walrus allows only one sem wait per instruction; hoist extras
    onto same-engine NoOps inserted immediately before."""
    bir = json.loads(bir_bytes)
    n = [0]
    changed = False
    for func in bir.get("functions", []):
        for bb in func.get("blocks", []):
            out = []
            for inst in bb.get("instructions", []):
                si = inst.get("sync_info")
                if si:
                    upds = si.get("on_update") or []
                    assert len(upds) <= 1, (inst.get("name"), len(upds))
                waits = (si or {}).get("on_wait") or []
                if len(waits) > 1:
                    changed = True
                    for w in waits[:-1]:
                        n[0] += 1
                        out.append({
                            "debug": inst.get("debug", 0),
                            "engine": inst["engine"],
                            "ins": [],
                            "name": f"I-waitsplit-{n[0]}",
                            "opcode": "NoOp",
                            "outs": [],
                            "sync_info": {"on_update": [], "on_wait": [w]},
                        })
                    si["on_wait"] = [waits[-1]]
                out.append(inst)
            bb["instructions"] = out
    return json.dumps(bir).encode() if changed else bir_bytes


def _install_patch():
    import concourse.bass2jax as b2j
    import concourse.bass_utils as bu

    if getattr(bu, "_waitsplit_patched", False):
        return
    orig = bu.compile_bir_kernel

    def patched(bir_json, tmpdir, neff_name="file.neff"):
        if isinstance(bir_json, str):
            bir_json = bir_json.encode()
        return orig(_legalize_bir_json(bir_json), tmpdir, neff_name=neff_name)

    b2j.compile_bir_kernel = patched
    bu.compile_bir_kernel = patched
    bu._waitsplit_patched = True


# ---------------------------------------------------------------- bass program
def build_nc():
    _install_patch()
    import concourse.bass as bass
    import concourse.mybir as mybir
    from concourse.tile import TileContext

    F32 = mybir.dt.float32
    F32R = mybir.dt.float32r
    AF = mybir.ActivationFunctionType
    OP = mybir.AluOpType

    nc = bass.Bass("TRN2")
    xT = nc.dram_tensor("xT", [C, T], F32R, kind="ExternalInput")
    w_qk = nc.dram_tensor("w_qk", [C, 1024], F32R, kind="ExternalInput")
    w_v = nc.dram_tensor("w_v", [C, 512], F32R, kind="ExternalInput")
    w_p = nc.dram_tensor("w_p", [512, 1024], F32R, kind="ExternalInput")
    b_qk = nc.dram_tensor("b_qk", [1, 1024], F32R, kind="ExternalInput")
    b_v = nc.dram_tensor("b_v", [1, 512], F32R, kind="ExternalInput")
    outT = nc.dram_tensor("outT", [C, T], F32, kind="ExternalOutput")

    xT_r = xT.rearrange("(co p) t -> p co t", p=128)      # [128, 8, T]
    wqk_r = w_qk.rearrange("(co p) n -> p co n", p=128)   # [128, 8, 1024]
    wv_r = w_v.rearrange("(co p) n -> p co n", p=128)     # [128, 8, 512]
    wp_r = w_p.rearrange("(fo p) n -> p fo n", p=128)     # [128, 4, 1024]
    outT_r = outT.rearrange("(mo p) t -> p mo t", p=128)  # [128, 8, T]

    with TileContext(nc) as tc:
        with tc.tile_pool(name="persist", bufs=1) as pp:
            qkT = pp.tile([128, 8, T], F32R)        # rows: q^T (0..3), k^T (4..7)
            vt = pp.tile([128, NKT, NH, D + 1], F32R)
            ones_f = pp.tile([1, TCH], F32)
            nc.gpsimd.memset(ones_f[:], 1.0)
            ones_r = pp.tile([1, TCH], F32R)
            nc.vector.tensor_copy(ones_r[:], ones_f[:])
            onesp_f = pp.tile([128, 1], F32)
            nc.gpsimd.memset(onesp_f[:], 1.0)
            onesrow_r = pp.tile([1, 128], F32R)
            nc.vector.tensor_copy(onesrow_r[:], onesp_f[0:1, :].to_broadcast([1, 128]))
            # ones column of v' tiles
            nc.vector.tensor_copy(
                vt[:, :, :, D:D + 1], onesp_f[:].to_broadcast([128, NKT, NH, 1])
            )
            bqk_t = pp.tile([1, 1024], F32R)
            bv_t = pp.tile([1, 512], F32R)
            nc.sync.dma_start(bqk_t[:], b_qk[:])
            nc.sync.dma_start(bv_t[:], b_v[:])

            # ---------------- phase 1: qkv projections ----------------
            with tc.tile_pool(name="p1", bufs=1) as p1, \
                 tc.tile_pool(name="p1x", bufs=2) as p1x, \
                 tc.tile_pool(name="ps1", bufs=2, space="PSUM") as ps1, \
                 tc.tile_pool(name="ps1v", bufs=2, space="PSUM") as ps1v:
                wqk_t = p1.tile([128, 8, 1024], F32R)
                wv_t = p1.tile([128, 8, 512], F32R)
                nc.sync.dma_start(wqk_t[:], wqk_r[:])
                nc.sync.dma_start(wv_t[:], wv_r[:])
                for tch in range(NCH):
                    xt = p1x.tile([128, 8, TCH], F32R)
                    nc.sync.dma_start(xt[:], xT_r[:, :, tch * TCH:(tch + 1) * TCH])
                    for mc in range(8):
                        pq = ps1.tile([128, TCH], F32)
                        for cc in range(8):
                            nc.tensor.matmul(
                                pq[:], wqk_t[:, cc, mc * 128:(mc + 1) * 128],
                                xt[:, cc, :], start=(cc == 0), stop=False)
                        nc.tensor.matmul(
                            pq[:], bqk_t[:, mc * 128:(mc + 1) * 128], ones_r[:],
                            start=False, stop=True)
                        nc.vector.tensor_copy(
                            qkT[:, mc, tch * TCH:(tch + 1) * TCH], pq[:])
                    for tt in range(4):
                        pv = ps1v.tile([128, 512], F32)
                        for cc in range(8):
                            nc.tensor.matmul(
                                pv[:], xt[:, cc, tt * 128:(tt + 1) * 128],
                                wv_t[:, cc, :], start=(cc == 0), stop=False)
                        nc.tensor.matmul(
                            pv[:], onesrow_r[:], bv_t[:], start=False, stop=True)
                        nc.vector.tensor_copy(
                            vt[:, tch * 4 + tt, :, 0:D],
                            pv[:].rearrange("p (h d) -> p h d", h=NH))

            # ---------------- phase 2: attention ----------------
            with tc.tile_pool(name="persist2", bufs=1) as pp2:
                attnT = pp2.tile([128, 4, T], F32R)
                masks = []
                for j in range(4):
                    mk = pp2.tile([128, TCH], F32, tag=f"mask{j}")
                    nc.gpsimd.memset(mk[:], 1.0)
                    nc.gpsimd.affine_select(
                        out=mk[:], in_=mk[:], compare_op=OP.is_ge, fill=0.0,
                        base=-128 * j, pattern=[[1, TCH]], channel_multiplier=-1)
                    masks.append(mk)

                with tc.tile_pool(name="p2", bufs=3) as p2, \
                     tc.tile_pool(name="p2r", bufs=2) as p2r, \
                     tc.tile_pool(name="ps_st", bufs=4, space="PSUM") as ps_st, \
                     tc.tile_pool(name="ps_ot", bufs=1, space="PSUM") as ps_ot, \
                     tc.tile_pool(name="ps_bc", bufs=2, space="PSUM") as ps_bc:
                    for hp in range(4):
                        for ch in range(NCH):
                            nkt = 4 * ch + 4   # k-tiles needed (causal)
                            po = [ps_ot.tile([D + 1, TCH], F32, tag=f"ot{i}",
                                             name=f"ot{i}")
                                  for i in range(2)]
                            q0 = ch * TCH
                            for kt in range(nkt):
                                for i in range(2):
                                    h = 2 * hp + i
                                    prow = (h % 2) * 64
                                    st = ps_st.tile([128, TCH], F32, tag="st")
                                    nc.tensor.matmul(
                                        st[:],
                                        qkT[prow:prow + 64, 4 + hp,
                                            kt * 128:(kt + 1) * 128],
                                        qkT[prow:prow + 64, hp, q0:q0 + TCH],
                                        start=True, stop=True)
                                    pt = p2.tile([128, TCH], F32R, tag="pt")
                                    nc.scalar.activation(pt[:], st[:], AF.Exp,
                                                         scale=0.125)
                                    if kt >= 4 * ch:
                                        nc.vector.tensor_tensor(
                                            pt[:], pt[:], masks[kt - 4 * ch][:],
                                            OP.mult)
                                    nc.tensor.matmul(
                                        po[i][:], vt[:, kt, h, :], pt[:],
                                        start=(kt == 0), stop=(kt == nkt - 1))
                            for i in range(2):
                                h = 2 * hp + i
                                rc = p2r.tile([1, TCH], F32R, tag="rc")
                                with nc.allow_low_precision(
                                        reason="softmax denom recip"):
                                    nc.vector.reciprocal(rc[:], po[i][D:D + 1, :])
                                bc = ps_bc.tile([64, TCH], F32, tag="bc")
                                nc.tensor.matmul(
                                    bc[:], onesrow_r[:, 0:64], rc[:],
                                    start=True, stop=True)
                                bc_sb = p2r.tile([64, TCH], F32, tag="bc_sb")
                                nc.vector.tensor_copy(bc_sb[:], bc[:])
                                prow = (h % 2) * 64
                                nc.vector.tensor_tensor(
                                    attnT[prow:prow + 64, hp, q0:q0 + TCH],
                                    po[i][0:D, :], bc_sb[:], OP.mult)

                # ---------------- phase 3: output projection ----------------
                with tc.tile_pool(name="p3", bufs=1) as p3, \
                     tc.tile_pool(name="p3o", bufs=3) as p3o, \
                     tc.tile_pool(name="ps3", bufs=2, space="PSUM") as ps3:
                    wp_t = p3.tile([128, 4, 1024], F32R)
                    nc.sync.dma_start(wp_t[:], wp_r[:])
                    for mc in range(8):
                        for tch in range(NCH):
                            pj = ps3.tile([128, TCH], F32, tag="pj")
                            for fc in range(4):
                                nc.tensor.matmul(
                                    pj[:], wp_t[:, fc, mc * 128:(mc + 1) * 128],
                                    attnT[:, fc, tch * TCH:(tch + 1) * TCH],
                                    start=(fc == 0), stop=(fc == 3))
                            ob = p3o.tile([128, TCH], F32, tag="ob")
                            nc.vector.tensor_copy(ob[:], pj[:])
                            nc.sync.dma_start(
                                outT_r[:, mc, tch * TCH:(tch + 1) * TCH], ob[:])
    return nc


# ---------------------------------------------------------------- cached runner
def _make_runner():
    """Build nc once and return a callable(in_maps) -> list of out dicts,
    with the jitted sharded executable cached across calls."""
    import jax
    from jax.experimental.shard_map import shard_map
    from jax.sharding import Mesh, PartitionSpec

    import concourse.mybir as mybir
    from concourse import bass2jax

    nc = build_nc()

    partition_name = (nc.partition_id_tensor.name
                      if nc.partition_id_tensor else None)
    in_names = []
    out_names = []
    out_avals = []
    zero_shapes = []
    for alloc in nc.m.functions[0].allocations:
        if not isinstance(alloc, mybir.MemoryLocationSet):
            continue
        name = alloc.memorylocations[0].name
        if alloc.kind == "ExternalInput":
            if name != partition_name:
                in_names.append(name)
        elif alloc.kind == "ExternalOutput":
            out_names.append(name)
            shape = tuple(alloc.tensor_shape)
            dtype = mybir.dt.np(alloc.dtype)
            out_avals.append(jax.core.ShapedArray(shape, dtype))
            zero_shapes.append((shape, dtype))
    n_params = len(in_names)
    all_names = list(in_names + out_names)
    if partition_name is not None:
        all_names.append(partition_name)
    all_names = tuple(all_names)

    def _body(*args):
        operands = list(args)
        if partition_name is not None:
            operands.append(bass2jax.partition_id_tensor())
        outs = bass2jax._bass_exec_p.bind(
            *operands,
            out_avals=tuple(out_avals),
            in_names=all_names,
            out_names=tuple(out_names),
            lowering_input_output_aliases=(),
            sim_require_finite=True,
            sim_require_nnan=True,
            nc=nc,
        )
        return tuple(outs)

    devices = jax.devices()[:8]
    mesh = Mesh(np.asarray(devices), ("core",))
    n_outs = len(out_names)
    sharded = jax.jit(
        shard_map(
            _body, mesh=mesh,
            in_specs=(PartitionSpec("core"),) * (n_params + n_outs),
            out_specs=(PartitionSpec("core"),) * n_outs,
            check_rep=False,
        ),
        donate_argnums=tuple(range(n_params, n_params + n_outs)),
        keep_unused=True,
    )

    def run(in_maps):
        concat_in = [
            np.concatenate([np.asarray(m[name]) for m in in_maps], axis=0)
            for name in in_names
        ]
        concat_zeros = [
            np.zeros((8 * s[0], *s[1:]), dt) for s, dt in zero_shapes
        ]
        out_arrs = sharded(*concat_in, *concat_zeros)
        return [
            {
                name: np.asarray(out_arrs[i]).reshape(8, *zero_shapes[i][0])[c]
                for i, name in enumerate(out_names)
            }
            for c in range(8)
        ]

    return run


# ---------------------------------------------------------------- host wrapper
def make_in_maps(x, w_qkv, b_qkv, w_proj, b_proj):
    in_maps = []
    for b in range(B):
        xT_b = np.ascontiguousarray(x[b].T, dtype=np.float32)
        for hg in range(2):
            s = hg * 512
            wqk = np.ascontiguousarray(
                np.concatenate(
                    [w_qkv[:, s:s + 512], w_qkv[:, 1024 + s:1024 + s + 512]],
                    axis=1), dtype=np.float32)
            wv = np.ascontiguousarray(w_qkv[:, 2048 + s:2048 + s + 512],
                                      dtype=np.float32)
            wp = np.ascontiguousarray(w_proj[s:s + 512, :], dtype=np.float32)
            bqk = np.concatenate(
                [b_qkv[s:s + 512], b_qkv[1024 + s:1024 + s + 512]]
            ).reshape(1, 1024).astype(np.float32)
            bv = b_qkv[2048 + s:2048 + s + 512].reshape(1, 512).astype(np.float32)
            in_maps.append({"xT": xT_b, "w_qk": wqk, "w_v": wv, "w_p": wp,
                            "b_qk": bqk, "b_v": bv})
    return in_maps


def combine(results, b_proj):
    out = np.empty((B, T, C), dtype=np.float32)
    for b in range(B):
        acc = results[2 * b]["outT"] + results[2 * b + 1]["outT"]  # [1024, T]
        out[b] = acc.T
    out += b_proj.astype(np.float32)
    return out


def kernel(x, w_qkv, b_qkv, w_proj, b_proj):
    global _RUNNER
    if _RUNNER is None:
        _RUNNER = _make_runner()
    in_maps = make_in_maps(np.asarray(x), np.asarray(w_qkv), np.asarray(b_qkv),
                           np.asarray(w_proj), np.asarray(b_proj))
    results = _RUNNER(in_maps)
    return combine(results, np.asarray(b_proj))


# revision 17
# speedup vs baseline: 1.3135x; 1.3135x over previous
"""Causal self-attention (B=4, T=2048, D=1024, H=16, d=64) on 8 TRN2 cores.

Sharding: 8 cores = 4 batches x 2 head-groups (8 heads each). Each core
computes, for its (batch, head-group):
  qk^T = (x @ w_qk)^T           [1024, T]   (q^T rows 0..511, k^T rows 512..1023)
  v    = x @ w_v                [T, 512]    (+ ones column per head -> [.., 65])
  S^T  = K^T.T @ Q^T per head   [k, q] tiles, exp via ACT (scale 1/8 folded)
  P^T causal-masked, AV: oT[65, q] accumulates V'.T @ P^T  (row 64 = denom)
  attn^T = oT[0:64] * (1/denom) (denominator broadcast via K=1 matmul)
  out^T += w_proj_slice.T-chunks @ attn^T  -> [1024, T] partial
Host sums the two head-group partials per batch, adds biases' linear terms,
and transposes back. All matmuls run as float32r (full-rate fp32).
"""

import json

import numpy as np

B = 4
T = 2048
C = 1024
NH = 8          # heads per core
D = 64
TCH = 512       # q/time chunk
NKT = T // 128  # 16 k-tiles
NCH = T // TCH  # 4 chunks

_RUNNER = None


# ---------------------------------------------------------------- BIR legalize
def _legalize_bir_json(bir_bytes):
    """Stock walrus allows only one sem wait per instruction; hoist extras
    onto same-engine NoOps inserted immediately before."""
    bir = json.loads(bir_bytes)
    n = [0]
    changed = False
    for func in bir.get("functions", []):
        for bb in func.get("blocks", []):
            out = []
            for inst in bb.get("instructions", []):
                si = inst.get("sync_info")
                if si:
                    upds = si.get("on_update") or []
                    assert len(upds) <= 1, (inst.get("name"), len(upds))
                waits = (si or {}).get("on_wait") or []
                if len(waits) > 1:
                    changed = True
                    for w in waits[:-1]:
                        n[0] += 1
                        out.append({
                            "debug": inst.get("debug", 0),
                            "engine": inst["engine"],
                            "ins": [],
                            "name": f"I-waitsplit-{n[0]}",
                            "opcode": "NoOp",
                            "outs": [],
                            "sync_info": {"on_update": [], "on_wait": [w]},
                        })
                    si["on_wait"] = [waits[-1]]
                out.append(inst)
            bb["instructions"] = out
    return json.dumps(bir).encode() if changed else bir_bytes


def _install_patch():
    import concourse.bass2jax as b2j
    import concourse.bass_utils as bu

    if getattr(bu, "_waitsplit_patched", False):
        return
    orig = bu.compile_bir_kernel

    def patched(bir_json, tmpdir, neff_name="file.neff"):
        if isinstance(bir_json, str):
            bir_json = bir_json.encode()
        return orig(_legalize_bir_json(bir_json), tmpdir, neff_name=neff_name)

    b2j.compile_bir_kernel = patched
    bu.compile_bir_kernel = patched
    bu._waitsplit_patched = True


# ---------------------------------------------------------------- bass program
def build_nc(with_bias=False):
    _install_patch()
    import concourse.bass as bass
    import concourse.mybir as mybir
    from concourse.tile import TileContext

    F32 = mybir.dt.float32
    F32R = mybir.dt.float32r
    AF = mybir.ActivationFunctionType
    OP = mybir.AluOpType

    nc = bass.Bass("TRN2")
    xT = nc.dram_tensor("xT", [C, T], F32R, kind="ExternalInput")
    w_qk = nc.dram_tensor("w_qk", [C, 1024], F32R, kind="ExternalInput")
    w_v = nc.dram_tensor("w_v", [C, 512], F32R, kind="ExternalInput")
    w_p = nc.dram_tensor("w_p", [512, 1024], F32R, kind="ExternalInput")
    b_qk = nc.dram_tensor("b_qk", [1, 1024], F32R, kind="ExternalInput")
    b_v = nc.dram_tensor("b_v", [1, 512], F32R, kind="ExternalInput")
    outT = nc.dram_tensor("outT", [C, T], F32, kind="ExternalOutput")

    xT_r = xT.rearrange("(co p) t -> p co t", p=128)      # [128, 8, T]
    wqk_r = w_qk.rearrange("(co p) n -> p co n", p=128)   # [128, 8, 1024]
    wv_r = w_v.rearrange("(co p) n -> p co n", p=128)     # [128, 8, 512]
    wp_r = w_p.rearrange("(fo p) n -> p fo n", p=128)     # [128, 4, 1024]
    outT_r = outT.rearrange("(mo p) t -> p mo t", p=128)  # [128, 8, T]

    with TileContext(nc) as tc:
        with tc.tile_pool(name="persist", bufs=1) as pp:
            qkT = pp.tile([128, 8, T], F32R)        # rows: q^T (0..3), k^T (4..7)
            vt = pp.tile([128, NKT, NH, D + 1], F32R)
            onesp_f = pp.tile([128, 1], F32)
            nc.gpsimd.memset(onesp_f[:], 1.0)
            onesrow_r = pp.tile([1, 128], F32R)
            nc.vector.tensor_copy(onesrow_r[:], onesp_f[0:1, :].to_broadcast([1, 128]))
            # ones column of v' tiles
            nc.vector.tensor_copy(
                vt[:, :, :, D:D + 1], onesp_f[:].to_broadcast([128, NKT, NH, 1])
            )
            bqk_t = pp.tile([1, 1024], F32R)
            bv_t = pp.tile([1, 512], F32R)
            nc.sync.dma_start(bqk_t[:], b_qk[:])
            nc.sync.dma_start(bv_t[:], b_v[:])
            if with_bias:
                ones_f = pp.tile([1, TCH], F32)
                nc.gpsimd.memset(ones_f[:], 1.0)
                ones_r = pp.tile([1, TCH], F32R)
                nc.vector.tensor_copy(ones_r[:], ones_f[:])

            # ---------------- phase 1: qkv projections ----------------
            with tc.tile_pool(name="p1", bufs=1) as p1, \
                 tc.tile_pool(name="p1x", bufs=2) as p1x, \
                 tc.tile_pool(name="ps1", bufs=2, space="PSUM") as ps1, \
                 tc.tile_pool(name="ps1v", bufs=2, space="PSUM") as ps1v:
                wqk_t = p1.tile([128, 8, 1024], F32R)
                wv_t = p1.tile([128, 8, 512], F32R)
                nc.sync.dma_start(wqk_t[:], wqk_r[:])
                nc.sync.dma_start(wv_t[:], wv_r[:])
                for tch in range(NCH):
                    xt = p1x.tile([128, 8, TCH], F32R)
                    nc.sync.dma_start(xt[:], xT_r[:, :, tch * TCH:(tch + 1) * TCH])
                    for mc in range(8):
                        pq = ps1.tile([128, TCH], F32)
                        for cc in range(8):
                            nc.tensor.matmul(
                                pq[:], wqk_t[:, cc, mc * 128:(mc + 1) * 128],
                                xt[:, cc, :], start=(cc == 0),
                                stop=(cc == 7 and not with_bias))
                        if with_bias:
                            nc.tensor.matmul(
                                pq[:], bqk_t[:, mc * 128:(mc + 1) * 128],
                                ones_r[:], start=False, stop=True)
                        nc.vector.tensor_copy(
                            qkT[:, mc, tch * TCH:(tch + 1) * TCH], pq[:])
                    for tt in range(4):
                        pv = ps1v.tile([128, 512], F32)
                        for cc in range(8):
                            nc.tensor.matmul(
                                pv[:], xt[:, cc, tt * 128:(tt + 1) * 128],
                                wv_t[:, cc, :], start=(cc == 0),
                                stop=(cc == 7 and not with_bias))
                        if with_bias:
                            nc.tensor.matmul(
                                pv[:], onesrow_r[:], bv_t[:],
                                start=False, stop=True)
                        nc.vector.tensor_copy(
                            vt[:, tch * 4 + tt, :, 0:D],
                            pv[:].rearrange("p (h d) -> p h d", h=NH))

            # ---------------- phase 2: attention ----------------
            with tc.tile_pool(name="persist2", bufs=1) as pp2:
                attnT = pp2.tile([128, 4, T], F32R)
                masks = []
                for j in range(4):
                    mk = pp2.tile([128, 2 * TCH], F32, tag=f"mask{j}",
                                  name=f"mask{j}")
                    nc.gpsimd.memset(mk[:], 1.0)
                    for half in range(2):
                        nc.gpsimd.affine_select(
                            out=mk[:, half * TCH:(half + 1) * TCH],
                            in_=mk[:, half * TCH:(half + 1) * TCH],
                            compare_op=OP.is_ge, fill=0.0,
                            base=-128 * j, pattern=[[1, TCH]],
                            channel_multiplier=-1)
                    masks.append(mk)

                with tc.tile_pool(name="p2", bufs=3) as p2, \
                     tc.tile_pool(name="p2r", bufs=2) as p2r, \
                     tc.tile_pool(name="ps_st", bufs=2, space="PSUM") as ps_st, \
                     tc.tile_pool(name="ps_ot", bufs=1, space="PSUM") as ps_ot, \
                     tc.tile_pool(name="ps_bc", bufs=2, space="PSUM") as ps_bc:
                    for ch in range(NCH):
                        nkt = 4 * ch + 4   # k-tiles needed (causal)
                        q0 = ch * TCH
                        for hp in range(4):
                            po = [ps_ot.tile([D + 1, TCH], F32, tag=f"ot{i}",
                                             name=f"ot{i}")
                                  for i in range(2)]
                            for kt in range(nkt):
                                stp = ps_st.tile([128, 2 * TCH], F32, tag="st",
                                                 name="stp")
                                for i in range(2):
                                    prow = i * 64
                                    nc.tensor.matmul(
                                        stp[:, i * TCH:(i + 1) * TCH],
                                        qkT[prow:prow + 64, 4 + hp,
                                            kt * 128:(kt + 1) * 128],
                                        qkT[prow:prow + 64, hp, q0:q0 + TCH],
                                        start=True, stop=True)
                                ptp = p2.tile([128, 2 * TCH], F32R, tag="pt",
                                              name="ptp")
                                nc.scalar.activation(ptp[:], stp[:], AF.Exp,
                                                     scale=0.125)
                                if kt >= 4 * ch:
                                    nc.vector.tensor_tensor(
                                        ptp[:], ptp[:], masks[kt - 4 * ch][:],
                                        OP.mult)
                                for i in range(2):
                                    nc.tensor.matmul(
                                        po[i][:], vt[:, kt, 2 * hp + i, :],
                                        ptp[:, i * TCH:(i + 1) * TCH],
                                        start=(kt == 0), stop=(kt == nkt - 1))
                            for i in range(2):
                                h = 2 * hp + i
                                rc = p2r.tile([1, TCH], F32R, tag="rc")
                                with nc.allow_low_precision(
                                        reason="softmax denom recip"):
                                    nc.vector.reciprocal(rc[:], po[i][D:D + 1, :])
                                bc = ps_bc.tile([64, TCH], F32, tag="bc")
                                nc.tensor.matmul(
                                    bc[:], onesrow_r[:, 0:64], rc[:],
                                    start=True, stop=True)
                                bc_sb = p2r.tile([64, TCH], F32, tag="bc_sb")
                                nc.vector.tensor_copy(bc_sb[:], bc[:])
                                prow = (h % 2) * 64
                                nc.vector.tensor_tensor(
                                    attnT[prow:prow + 64, hp, q0:q0 + TCH],
                                    po[i][0:D, :], bc_sb[:], OP.mult)

                # ---------------- phase 3: output projection ----------------
                with tc.tile_pool(name="p3", bufs=1) as p3, \
                     tc.tile_pool(name="p3o", bufs=3) as p3o, \
                     tc.tile_pool(name="ps3", bufs=2, space="PSUM") as ps3:
                    wp_t = p3.tile([128, 4, 1024], F32R)
                    nc.sync.dma_start(wp_t[:], wp_r[:])
                    for mc in range(8):
                        for tch in range(NCH):
                            pj = ps3.tile([128, TCH], F32, tag="pj")
                            for fc in range(4):
                                nc.tensor.matmul(
                                    pj[:], wp_t[:, fc, mc * 128:(mc + 1) * 128],
                                    attnT[:, fc, tch * TCH:(tch + 1) * TCH],
                                    start=(fc == 0), stop=(fc == 3))
                            ob = p3o.tile([128, TCH], F32, tag="ob")
                            nc.vector.tensor_copy(ob[:], pj[:])
                            nc.sync.dma_start(
                                outT_r[:, mc, tch * TCH:(tch + 1) * TCH], ob[:])
    return nc


# ---------------------------------------------------------------- cached runner
def _make_runner(with_bias=False):
    """Build nc once and return a callable(in_maps) -> list of out dicts,
    with the jitted sharded executable cached across calls."""
    import jax
    from jax.experimental.shard_map import shard_map
    from jax.sharding import Mesh, PartitionSpec

    import concourse.mybir as mybir
    from concourse import bass2jax

    nc = build_nc(with_bias=with_bias)

    partition_name = (nc.partition_id_tensor.name
                      if nc.partition_id_tensor else None)
    in_names = []
    out_names = []
    out_avals = []
    zero_shapes = []
    for alloc in nc.m.functions[0].allocations:
        if not isinstance(alloc, mybir.MemoryLocationSet):
            continue
        name = alloc.memorylocations[0].name
        if alloc.kind == "ExternalInput":
            if name != partition_name:
                in_names.append(name)
        elif alloc.kind == "ExternalOutput":
            out_names.append(name)
            shape = tuple(alloc.tensor_shape)
            dtype = mybir.dt.np(alloc.dtype)
            out_avals.append(jax.core.ShapedArray(shape, dtype))
            zero_shapes.append((shape, dtype))
    n_params = len(in_names)
    all_names = list(in_names + out_names)
    if partition_name is not None:
        all_names.append(partition_name)
    all_names = tuple(all_names)

    def _body(*args):
        operands = list(args)
        if partition_name is not None:
            operands.append(bass2jax.partition_id_tensor())
        outs = bass2jax._bass_exec_p.bind(
            *operands,
            out_avals=tuple(out_avals),
            in_names=all_names,
            out_names=tuple(out_names),
            lowering_input_output_aliases=(),
            sim_require_finite=True,
            sim_require_nnan=True,
            nc=nc,
        )
        return tuple(outs)

    devices = jax.devices()[:8]
    mesh = Mesh(np.asarray(devices), ("core",))
    n_outs = len(out_names)
    sharded = jax.jit(
        shard_map(
            _body, mesh=mesh,
            in_specs=(PartitionSpec("core"),) * (n_params + n_outs),
            out_specs=(PartitionSpec("core"),) * n_outs,
            check_rep=False,
        ),
        donate_argnums=tuple(range(n_params, n_params + n_outs)),
        keep_unused=True,
    )

    def run(in_maps):
        concat_in = [
            np.concatenate([np.asarray(m[name]) for m in in_maps], axis=0)
            for name in in_names
        ]
        concat_zeros = [
            np.zeros((8 * s[0], *s[1:]), dt) for s, dt in zero_shapes
        ]
        out_arrs = sharded(*concat_in, *concat_zeros)
        return [
            {
                name: np.asarray(out_arrs[i]).reshape(8, *zero_shapes[i][0])[c]
                for i, name in enumerate(out_names)
            }
            for c in range(8)
        ]

    return run


# ---------------------------------------------------------------- host wrapper
def make_in_maps(x, w_qkv, b_qkv, w_proj, b_proj):
    in_maps = []
    for b in range(B):
        xT_b = np.ascontiguousarray(x[b].T, dtype=np.float32)
        for hg in range(2):
            s = hg * 512
            wqk = np.ascontiguousarray(
                np.concatenate(
                    [w_qkv[:, s:s + 512], w_qkv[:, 1024 + s:1024 + s + 512]],
                    axis=1), dtype=np.float32)
            wv = np.ascontiguousarray(w_qkv[:, 2048 + s:2048 + s + 512],
                                      dtype=np.float32)
            wp = np.ascontiguousarray(w_proj[s:s + 512, :], dtype=np.float32)
            bqk = np.concatenate(
                [b_qkv[s:s + 512], b_qkv[1024 + s:1024 + s + 512]]
            ).reshape(1, 1024).astype(np.float32)
            bv = b_qkv[2048 + s:2048 + s + 512].reshape(1, 512).astype(np.float32)
            in_maps.append({"xT": xT_b, "w_qk": wqk, "w_v": wv, "w_p": wp,
                            "b_qk": bqk, "b_v": bv})
    return in_maps


def combine(results, b_proj):
    out = np.empty((B, T, C), dtype=np.float32)
    for b in range(B):
        acc = results[2 * b]["outT"] + results[2 * b + 1]["outT"]  # [1024, T]
        out[b] = acc.T
    out += b_proj.astype(np.float32)
    return out


def kernel(x, w_qkv, b_qkv, w_proj, b_proj):
    global _RUNNER
    b_qkv = np.asarray(b_qkv)
    with_bias = bool(np.any(b_qkv))
    if _RUNNER is None or _RUNNER[0] != with_bias:
        _RUNNER = (with_bias, _make_runner(with_bias=with_bias))
    in_maps = make_in_maps(np.asarray(x), np.asarray(w_qkv), b_qkv,
                           np.asarray(w_proj), np.asarray(b_proj))
    results = _RUNNER[1](in_maps)
    return combine(results, np.asarray(b_proj))


# revision 36
# speedup vs baseline: 1.3929x; 1.0605x over previous
"""Causal self-attention (B=4, T=2048, D=1024, H=16, d=64) on 8 TRN2 cores.

Sharding: 8 cores = 4 batches x 2 head-groups (8 heads each). Each core
computes, for its (batch, head-group):
  qk^T = (x @ w_qk)^T           [1024, T]   (q^T rows 0..511, k^T rows 512..1023)
  v    = x @ w_v                [T, 512]    (+ ones column per head -> [.., 65])
  S^T  = K^T.T @ Q^T per head   [k, q] tiles, exp via ACT (scale 1/8 folded)
  P^T causal-masked, AV: oT[65, q] accumulates V'.T @ P^T  (row 64 = denom)
  attn^T = oT[0:64] * (1/denom) (denominator broadcast via K=1 matmul)
  out^T += w_proj_slice.T-chunks @ attn^T  -> [1024, T] partial
Host sums the two head-group partials per batch, adds biases' linear terms,
and transposes back. All matmuls run as float32r (full-rate fp32).
"""

import json

import numpy as np

B = 4
T = 2048
C = 1024
NH = 8          # heads per core
D = 64
TCH = 512       # q/time chunk
NKT = T // 128  # 16 k-tiles
NCH = T // TCH  # 4 chunks

_RUNNER = None


# ---------------------------------------------------------------- BIR legalize
def _legalize_bir_json(bir_bytes):
    """Stock walrus allows only one sem wait per instruction; hoist extras
    onto same-engine NoOps inserted immediately before."""
    bir = json.loads(bir_bytes)
    n = [0]
    changed = False
    for func in bir.get("functions", []):
        for bb in func.get("blocks", []):
            out = []
            for inst in bb.get("instructions", []):
                si = inst.get("sync_info")
                if si:
                    upds = si.get("on_update") or []
                    assert len(upds) <= 1, (inst.get("name"), len(upds))
                waits = (si or {}).get("on_wait") or []
                if len(waits) > 1:
                    changed = True
                    for w in waits[:-1]:
                        n[0] += 1
                        out.append({
                            "debug": inst.get("debug", 0),
                            "engine": inst["engine"],
                            "ins": [],
                            "name": f"I-waitsplit-{n[0]}",
                            "opcode": "NoOp",
                            "outs": [],
                            "sync_info": {"on_update": [], "on_wait": [w]},
                        })
                    si["on_wait"] = [waits[-1]]
                out.append(inst)
            bb["instructions"] = out
    return json.dumps(bir).encode() if changed else bir_bytes


def _install_patch():
    import concourse.bass2jax as b2j
    import concourse.bass_utils as bu

    if getattr(bu, "_waitsplit_patched", False):
        return
    orig = bu.compile_bir_kernel

    def patched(bir_json, tmpdir, neff_name="file.neff"):
        if isinstance(bir_json, str):
            bir_json = bir_json.encode()
        return orig(_legalize_bir_json(bir_json), tmpdir, neff_name=neff_name)

    b2j.compile_bir_kernel = patched
    bu.compile_bir_kernel = patched
    bu._waitsplit_patched = True


# ---------------------------------------------------------------- bass program
def build_nc(with_bias=False):
    _install_patch()
    import concourse.bass as bass
    import concourse.mybir as mybir
    from concourse.tile import TileContext

    F32 = mybir.dt.float32
    F32R = mybir.dt.float32r
    AF = mybir.ActivationFunctionType
    OP = mybir.AluOpType

    nc = bass.Bass("TRN2")
    xT = nc.dram_tensor("xT", [C, T], F32R, kind="ExternalInput")
    w_qk = nc.dram_tensor("w_qk", [C, 1024], F32R, kind="ExternalInput")
    w_v = nc.dram_tensor("w_v", [C, 512], F32R, kind="ExternalInput")
    w_p = nc.dram_tensor("w_p", [512, 1024], F32R, kind="ExternalInput")
    b_qk = nc.dram_tensor("b_qk", [1, 1024], F32R, kind="ExternalInput")
    b_v = nc.dram_tensor("b_v", [1, 512], F32R, kind="ExternalInput")
    outT = nc.dram_tensor("outT", [C, T], F32, kind="ExternalOutput")

    xT_r = xT.rearrange("(co p) t -> p co t", p=128)      # [128, 8, T]
    wqk_r = w_qk.rearrange("(co p) n -> p co n", p=128)   # [128, 8, 1024]
    wv_r = w_v.rearrange("(co p) n -> p co n", p=128)     # [128, 8, 512]
    wp_r = w_p.rearrange("(fo p) n -> p fo n", p=128)     # [128, 4, 1024]
    outT_r = outT.rearrange("(mo p) t -> p mo t", p=128)  # [128, 8, T]

    with TileContext(nc) as tc:
        with tc.tile_pool(name="persist", bufs=1) as pp:
            qkT = pp.tile([128, 8, T], F32R)        # rows: q^T (0..3), k^T (4..7)
            # v' per (k-tile, head): [v (64) | ones (1)] -> AV lhsT [128, 65];
            # PSUM row 64 accumulates the softmax denominator.
            vt = pp.tile([128, NKT, NH, D + 1], F32R)
            onesp_f = pp.tile([128, 1], F32)
            nc.gpsimd.memset(onesp_f[:], 1.0)
            nc.vector.tensor_copy(
                vt[:, :, :, D:D + 1], onesp_f[:].to_broadcast([128, NKT, NH, 1]))
            onesbc_r = pp.tile([1, D], F32R)
            nc.vector.tensor_copy(onesbc_r[:],
                                  onesp_f[0:1, :].to_broadcast([1, D]))
            bqk_t = pp.tile([1, 1024], F32R)
            bv_t = pp.tile([1, 512], F32R)
            nc.sync.dma_start(bqk_t[:], b_qk[:])
            nc.sync.dma_start(bv_t[:], b_v[:])
            if with_bias:
                ones_f = pp.tile([1, TCH], F32)
                nc.gpsimd.memset(ones_f[:], 1.0)
                ones_r = pp.tile([1, TCH], F32R)
                nc.vector.tensor_copy(ones_r[:], ones_f[:])
                onesrow_r = pp.tile([1, 128], F32R)
                nc.vector.tensor_copy(
                    onesrow_r[:], onesp_f[0:1, :].to_broadcast([1, 128]))

            # ---------------- phase 1: qkv projections ----------------
            with tc.tile_pool(name="p1", bufs=1) as p1, \
                 tc.tile_pool(name="p1x", bufs=2) as p1x, \
                 tc.tile_pool(name="ps1", bufs=2, space="PSUM") as ps1, \
                 tc.tile_pool(name="ps1v", bufs=2, space="PSUM") as ps1v:
                wqk_t = p1.tile([128, 8, 1024], F32R)
                wv_t = p1.tile([128, 8, 512], F32R)
                nc.sync.dma_start(wqk_t[:], wqk_r[:])
                nc.sync.dma_start(wv_t[:], wv_r[:])
                for tch in range(NCH):
                    xt = p1x.tile([128, 8, TCH], F32R)
                    nc.sync.dma_start(xt[:], xT_r[:, :, tch * TCH:(tch + 1) * TCH])
                    for mc in range(8):
                        pq = ps1.tile([128, TCH], F32)
                        for cc in range(8):
                            nc.tensor.matmul(
                                pq[:], wqk_t[:, cc, mc * 128:(mc + 1) * 128],
                                xt[:, cc, :], start=(cc == 0),
                                stop=(cc == 7 and not with_bias))
                        if with_bias:
                            nc.tensor.matmul(
                                pq[:], bqk_t[:, mc * 128:(mc + 1) * 128],
                                ones_r[:], start=False, stop=True)
                        nc.vector.tensor_copy(
                            qkT[:, mc, tch * TCH:(tch + 1) * TCH], pq[:])
                    for tt in range(4):
                        pv = ps1v.tile([128, 512], F32)
                        for cc in range(8):
                            nc.tensor.matmul(
                                pv[:], xt[:, cc, tt * 128:(tt + 1) * 128],
                                wv_t[:, cc, :], start=(cc == 0),
                                stop=(cc == 7 and not with_bias))
                        if with_bias:
                            nc.tensor.matmul(
                                pv[:], onesrow_r[:], bv_t[:],
                                start=False, stop=True)
                        nc.vector.tensor_copy(
                            vt[:, tch * 4 + tt, :, 0:D],
                            pv[:].rearrange("p (h d) -> p h d", h=NH))

            # ---------------- phase 2: attention ----------------
            with tc.tile_pool(name="persist2", bufs=1) as pp2:
                attnT = pp2.tile([128, 4, T], F32R)
                masks = []
                for j in range(4):
                    mk = pp2.tile([128, 2, TCH], F32, tag=f"mask{j}",
                                  name=f"mask{j}")
                    nc.gpsimd.memset(mk[:], 1.0)
                    for half in range(2):
                        nc.gpsimd.affine_select(
                            out=mk[:, half, :], in_=mk[:, half, :],
                            compare_op=OP.is_ge, fill=0.0,
                            base=-128 * j, pattern=[[1, TCH]],
                            channel_multiplier=-1)
                    masks.append(mk)

                with tc.tile_pool(name="p2", bufs=3) as p2, \
                     tc.tile_pool(name="p2r", bufs=3) as p2r, \
                     tc.tile_pool(name="ps_st", bufs=2, space="PSUM") as ps_st, \
                     tc.tile_pool(name="ps_ot", bufs=1, space="PSUM") as ps_ot, \
                     tc.tile_pool(name="ps_bc", bufs=2, space="PSUM") as ps_bc:
                    for ch in range(NCH):
                        nkt = 4 * ch + 4   # k-tiles needed (causal)
                        q0 = ch * TCH
                        for hp in range(4):
                            po = [ps_ot.tile([D + 1, TCH], F32, tag=f"ot{i}",
                                             name=f"ot{i}")
                                  for i in range(2)]
                            for kt in range(nkt):
                                stp = ps_st.tile([128, 2 * TCH], F32, tag="st",
                                                 name="stp")
                                for i in range(2):
                                    prow = i * 64
                                    nc.tensor.matmul(
                                        stp[:, i * TCH:(i + 1) * TCH],
                                        qkT[prow:prow + 64, 4 + hp,
                                            kt * 128:(kt + 1) * 128],
                                        qkT[prow:prow + 64, hp, q0:q0 + TCH],
                                        start=True, stop=True)
                                ptp = p2.tile([128, 2 * TCH], F32R, tag="pt",
                                              name="ptp")
                                nc.scalar.activation(ptp[:], stp[:], AF.Exp,
                                                     scale=0.125)
                                j = kt - 4 * ch
                                if j >= 0:
                                    w = (j + 1) * 128
                                    ptv = ptp[:].rearrange(
                                        "p (two t) -> p two t", two=2)
                                    mkv = masks[j]
                                    nc.vector.tensor_tensor(
                                        ptv[:, :, 0:w], ptv[:, :, 0:w],
                                        mkv[:, :, 0:w], OP.mult)
                                for i in range(2):
                                    nc.tensor.matmul(
                                        po[i][:],
                                        vt[:, kt, 2 * hp + i, :],
                                        ptp[:, i * TCH:(i + 1) * TCH],
                                        start=(kt == 0), stop=(kt == nkt - 1))
                            for i in range(2):
                                h = 2 * hp + i
                                po_sb = p2r.tile([D + 1, TCH], F32, tag="posb",
                                                 name="posb")
                                nc.scalar.copy(po_sb[:], po[i][:])
                                rc = p2r.tile([1, TCH], F32R, tag="rc")
                                with nc.allow_low_precision(
                                        reason="softmax denom recip"):
                                    nc.vector.reciprocal(rc[:],
                                                         po_sb[D:D + 1, :])
                                bc = ps_bc.tile([64, TCH], F32, tag="bc")
                                nc.tensor.matmul(
                                    bc[:], onesbc_r[:], rc[:],
                                    start=True, stop=True)
                                prow = (h % 2) * 64
                                nc.vector.tensor_tensor(
                                    attnT[prow:prow + 64, hp, q0:q0 + TCH],
                                    po_sb[0:D, :], bc[:], OP.mult)

                # ---------------- phase 3: output projection ----------------
                with tc.tile_pool(name="p3", bufs=1) as p3, \
                     tc.tile_pool(name="p3o", bufs=3) as p3o, \
                     tc.tile_pool(name="ps3", bufs=2, space="PSUM") as ps3:
                    wp_t = p3.tile([128, 4, 1024], F32R)
                    nc.sync.dma_start(wp_t[:], wp_r[:])
                    for mc in range(8):
                        for tch in range(NCH):
                            pj = ps3.tile([128, TCH], F32, tag="pj")
                            for fc in range(4):
                                nc.tensor.matmul(
                                    pj[:], wp_t[:, fc, mc * 128:(mc + 1) * 128],
                                    attnT[:, fc, tch * TCH:(tch + 1) * TCH],
                                    start=(fc == 0), stop=(fc == 3))
                            ob = p3o.tile([128, TCH], F32, tag="ob")
                            nc.vector.tensor_copy(ob[:], pj[:])
                            nc.sync.dma_start(
                                outT_r[:, mc, tch * TCH:(tch + 1) * TCH], ob[:])
    return nc


# ---------------------------------------------------------------- cached runner
def _make_runner(with_bias=False):
    """Build nc once and return a callable(in_maps) -> list of out dicts,
    with the jitted sharded executable cached across calls."""
    import jax
    from jax.experimental.shard_map import shard_map
    from jax.sharding import Mesh, PartitionSpec

    import concourse.mybir as mybir
    from concourse import bass2jax

    nc = build_nc(with_bias=with_bias)

    partition_name = (nc.partition_id_tensor.name
                      if nc.partition_id_tensor else None)
    in_names = []
    out_names = []
    out_avals = []
    zero_shapes = []
    for alloc in nc.m.functions[0].allocations:
        if not isinstance(alloc, mybir.MemoryLocationSet):
            continue
        name = alloc.memorylocations[0].name
        if alloc.kind == "ExternalInput":
            if name != partition_name:
                in_names.append(name)
        elif alloc.kind == "ExternalOutput":
            out_names.append(name)
            shape = tuple(alloc.tensor_shape)
            dtype = mybir.dt.np(alloc.dtype)
            out_avals.append(jax.core.ShapedArray(shape, dtype))
            zero_shapes.append((shape, dtype))
    n_params = len(in_names)
    all_names = list(in_names + out_names)
    if partition_name is not None:
        all_names.append(partition_name)
    all_names = tuple(all_names)

    def _body(*args):
        operands = list(args)
        if partition_name is not None:
            operands.append(bass2jax.partition_id_tensor())
        outs = bass2jax._bass_exec_p.bind(
            *operands,
            out_avals=tuple(out_avals),
            in_names=all_names,
            out_names=tuple(out_names),
            lowering_input_output_aliases=(),
            sim_require_finite=True,
            sim_require_nnan=True,
            nc=nc,
        )
        return tuple(outs)

    devices = jax.devices()[:8]
    mesh = Mesh(np.asarray(devices), ("core",))
    n_outs = len(out_names)
    sharded = jax.jit(
        shard_map(
            _body, mesh=mesh,
            in_specs=(PartitionSpec("core"),) * (n_params + n_outs),
            out_specs=(PartitionSpec("core"),) * n_outs,
            check_rep=False,
        ),
        donate_argnums=tuple(range(n_params, n_params + n_outs)),
        keep_unused=True,
    )

    def run(in_maps):
        concat_in = [
            np.concatenate([np.asarray(m[name]) for m in in_maps], axis=0)
            for name in in_names
        ]
        concat_zeros = [
            np.zeros((8 * s[0], *s[1:]), dt) for s, dt in zero_shapes
        ]
        out_arrs = sharded(*concat_in, *concat_zeros)
        return [
            {
                name: np.asarray(out_arrs[i]).reshape(8, *zero_shapes[i][0])[c]
                for i, name in enumerate(out_names)
            }
            for c in range(8)
        ]

    return run


# ---------------------------------------------------------------- host wrapper
def make_in_maps(x, w_qkv, b_qkv, w_proj, b_proj):
    in_maps = []
    for b in range(B):
        xT_b = np.ascontiguousarray(x[b].T, dtype=np.float32)
        for hg in range(2):
            s = hg * 512
            wqk = np.ascontiguousarray(
                np.concatenate(
                    [w_qkv[:, s:s + 512], w_qkv[:, 1024 + s:1024 + s + 512]],
                    axis=1), dtype=np.float32)
            wv = np.ascontiguousarray(w_qkv[:, 2048 + s:2048 + s + 512],
                                      dtype=np.float32)
            wp = np.ascontiguousarray(w_proj[s:s + 512, :], dtype=np.float32)
            bqk = np.concatenate(
                [b_qkv[s:s + 512], b_qkv[1024 + s:1024 + s + 512]]
            ).reshape(1, 1024).astype(np.float32)
            bv = b_qkv[2048 + s:2048 + s + 512].reshape(1, 512).astype(np.float32)
            in_maps.append({"xT": xT_b, "w_qk": wqk, "w_v": wv, "w_p": wp,
                            "b_qk": bqk, "b_v": bv})
    return in_maps


def combine(results, b_proj):
    out = np.empty((B, T, C), dtype=np.float32)
    for b in range(B):
        acc = results[2 * b]["outT"] + results[2 * b + 1]["outT"]  # [1024, T]
        out[b] = acc.T
    out += b_proj.astype(np.float32)
    return out


def kernel(x, w_qkv, b_qkv, w_proj, b_proj):
    global _RUNNER
    b_qkv = np.asarray(b_qkv)
    with_bias = bool(np.any(b_qkv))
    if _RUNNER is None or _RUNNER[0] != with_bias:
        _RUNNER = (with_bias, _make_runner(with_bias=with_bias))
    in_maps = make_in_maps(np.asarray(x), np.asarray(w_qkv), b_qkv,
                           np.asarray(w_proj), np.asarray(b_proj))
    results = _RUNNER[1](in_maps)
    return combine(results, np.asarray(b_proj))


# revision 37
# speedup vs baseline: 1.4145x; 1.0155x over previous
"""Causal self-attention (B=4, T=2048, D=1024, H=16, d=64) on 8 TRN2 cores.

Sharding: 8 cores = 4 batches x 2 head-groups (8 heads each). Each core
computes, for its (batch, head-group):
  qk^T = (x @ w_qk)^T           [1024, T]   (q^T rows 0..511, k^T rows 512..1023)
  v    = x @ w_v                [T, 512]    (+ ones column per head -> [.., 65])
  S^T  = K^T.T @ Q^T per head   [k, q] tiles, exp via ACT (scale 1/8 folded)
  P^T causal-masked, AV: oT[65, q] accumulates V'.T @ P^T  (row 64 = denom)
  attn^T = oT[0:64] * (1/denom) (denominator broadcast via K=1 matmul)
  out^T += w_proj_slice.T-chunks @ attn^T  -> [1024, T] partial
Host sums the two head-group partials per batch, adds biases' linear terms,
and transposes back. All matmuls run as float32r (full-rate fp32).
"""

import json

import numpy as np

B = 4
T = 2048
C = 1024
NH = 8          # heads per core
D = 64
TCH = 512       # q/time chunk
NKT = T // 128  # 16 k-tiles
NCH = T // TCH  # 4 chunks

_RUNNER = None


# ---------------------------------------------------------------- BIR legalize
def _legalize_bir_json(bir_bytes):
    """Stock walrus allows only one sem wait per instruction; hoist extras
    onto same-engine NoOps inserted immediately before."""
    bir = json.loads(bir_bytes)
    n = [0]
    changed = False
    for func in bir.get("functions", []):
        for bb in func.get("blocks", []):
            out = []
            for inst in bb.get("instructions", []):
                si = inst.get("sync_info")
                if si:
                    upds = si.get("on_update") or []
                    assert len(upds) <= 1, (inst.get("name"), len(upds))
                waits = (si or {}).get("on_wait") or []
                if len(waits) > 1:
                    changed = True
                    for w in waits[:-1]:
                        n[0] += 1
                        out.append({
                            "debug": inst.get("debug", 0),
                            "engine": inst["engine"],
                            "ins": [],
                            "name": f"I-waitsplit-{n[0]}",
                            "opcode": "NoOp",
                            "outs": [],
                            "sync_info": {"on_update": [], "on_wait": [w]},
                        })
                    si["on_wait"] = [waits[-1]]
                out.append(inst)
            bb["instructions"] = out
    return json.dumps(bir).encode() if changed else bir_bytes


def _install_patch():
    import concourse.bass2jax as b2j
    import concourse.bass_utils as bu

    if getattr(bu, "_waitsplit_patched", False):
        return
    orig = bu.compile_bir_kernel

    def patched(bir_json, tmpdir, neff_name="file.neff"):
        if isinstance(bir_json, str):
            bir_json = bir_json.encode()
        return orig(_legalize_bir_json(bir_json), tmpdir, neff_name=neff_name)

    b2j.compile_bir_kernel = patched
    bu.compile_bir_kernel = patched
    bu._waitsplit_patched = True


# ---------------------------------------------------------------- bass program
def build_nc(with_bias=False):
    _install_patch()
    import concourse.bass as bass
    import concourse.mybir as mybir
    from concourse.tile import TileContext

    F32 = mybir.dt.float32
    F32R = mybir.dt.float32r
    AF = mybir.ActivationFunctionType
    OP = mybir.AluOpType

    nc = bass.Bass("TRN2")
    xT = nc.dram_tensor("xT", [C, T], F32R, kind="ExternalInput")
    w_qk = nc.dram_tensor("w_qk", [C, 1024], F32R, kind="ExternalInput")
    w_v = nc.dram_tensor("w_v", [C, 512], F32R, kind="ExternalInput")
    w_p = nc.dram_tensor("w_p", [512, 1024], F32R, kind="ExternalInput")
    b_qk = nc.dram_tensor("b_qk", [1, 1024], F32R, kind="ExternalInput")
    b_v = nc.dram_tensor("b_v", [1, 512], F32R, kind="ExternalInput")
    outT = nc.dram_tensor("outT", [C, T], F32, kind="ExternalOutput")

    xT_r = xT.rearrange("(co p) t -> p co t", p=128)      # [128, 8, T]
    wqk_r = w_qk.rearrange("(co p) n -> p co n", p=128)   # [128, 8, 1024]
    wv_r = w_v.rearrange("(co p) n -> p co n", p=128)     # [128, 8, 512]
    wp_r = w_p.rearrange("(fo p) n -> p fo n", p=128)     # [128, 4, 1024]
    outT_r = outT.rearrange("(mo p) t -> p mo t", p=128)  # [128, 8, T]

    with TileContext(nc) as tc:
        with tc.tile_pool(name="persist", bufs=1) as pp:
            qkT = pp.tile([128, 8, T], F32R)        # rows: q^T (0..3), k^T (4..7)
            # v' per (k-tile, head): [v (64) | ones (1)] -> AV lhsT [128, 65];
            # PSUM row 64 accumulates the softmax denominator.
            vt = pp.tile([128, NKT, NH, D + 1], F32R)
            onesp_f = pp.tile([128, 1], F32)
            nc.gpsimd.memset(onesp_f[:], 1.0)
            nc.vector.tensor_copy(
                vt[:, :, :, D:D + 1], onesp_f[:].to_broadcast([128, NKT, NH, 1]))
            onesbc_r = pp.tile([1, D], F32R)
            nc.vector.tensor_copy(onesbc_r[:],
                                  onesp_f[0:1, :].to_broadcast([1, D]))
            bqk_t = pp.tile([1, 1024], F32R)
            bv_t = pp.tile([1, 512], F32R)
            nc.sync.dma_start(bqk_t[:], b_qk[:])
            nc.sync.dma_start(bv_t[:], b_v[:])
            if with_bias:
                ones_f = pp.tile([1, TCH], F32)
                nc.gpsimd.memset(ones_f[:], 1.0)
                ones_r = pp.tile([1, TCH], F32R)
                nc.vector.tensor_copy(ones_r[:], ones_f[:])
                onesrow_r = pp.tile([1, 128], F32R)
                nc.vector.tensor_copy(
                    onesrow_r[:], onesp_f[0:1, :].to_broadcast([1, 128]))

            # ---------------- phase 1: qkv projections ----------------
            with tc.tile_pool(name="p1", bufs=1) as p1, \
                 tc.tile_pool(name="p1x", bufs=2) as p1x, \
                 tc.tile_pool(name="ps1", bufs=2, space="PSUM") as ps1, \
                 tc.tile_pool(name="ps1v", bufs=2, space="PSUM") as ps1v:
                wqk_t = p1.tile([128, 8, 1024], F32R)
                wv_t = p1.tile([128, 8, 512], F32R)
                nc.sync.dma_start(wqk_t[:], wqk_r[:])
                nc.sync.dma_start(wv_t[:], wv_r[:])
                for tch in range(NCH):
                    xt = p1x.tile([128, 8, TCH], F32R)
                    nc.sync.dma_start(xt[:], xT_r[:, :, tch * TCH:(tch + 1) * TCH])
                    for mc in range(8):
                        pq = ps1.tile([128, TCH], F32)
                        for cc in range(8):
                            nc.tensor.matmul(
                                pq[:], wqk_t[:, cc, mc * 128:(mc + 1) * 128],
                                xt[:, cc, :], start=(cc == 0),
                                stop=(cc == 7 and not with_bias))
                        if with_bias:
                            nc.tensor.matmul(
                                pq[:], bqk_t[:, mc * 128:(mc + 1) * 128],
                                ones_r[:], start=False, stop=True)
                        nc.vector.tensor_copy(
                            qkT[:, mc, tch * TCH:(tch + 1) * TCH], pq[:])
                    for tt in range(4):
                        pv = ps1v.tile([128, 512], F32)
                        for cc in range(8):
                            nc.tensor.matmul(
                                pv[:], xt[:, cc, tt * 128:(tt + 1) * 128],
                                wv_t[:, cc, :], start=(cc == 0),
                                stop=(cc == 7 and not with_bias))
                        if with_bias:
                            nc.tensor.matmul(
                                pv[:], onesrow_r[:], bv_t[:],
                                start=False, stop=True)
                        nc.vector.tensor_copy(
                            vt[:, tch * 4 + tt, :, 0:D],
                            pv[:].rearrange("p (h d) -> p h d", h=NH))

            # ---------------- phase 2: attention ----------------
            with tc.tile_pool(name="persist2", bufs=1) as pp2:
                attnT = pp2.tile([128, 4, T], F32R)
                # prefetch proj weights so phase 3 starts without a DMA stall
                wp_t = pp2.tile([128, 4, 1024], F32R)
                nc.sync.dma_start(wp_t[:], wp_r[:])
                # single 128x128 lower-tri mask, duplicated for the head pair
                mask2 = pp2.tile([128, 2, 128], F32)
                nc.gpsimd.memset(mask2[:], 1.0)
                for half in range(2):
                    nc.gpsimd.affine_select(
                        out=mask2[:, half, :], in_=mask2[:, half, :],
                        compare_op=OP.is_ge, fill=0.0,
                        base=0, pattern=[[1, 128]], channel_multiplier=-1)

                with tc.tile_pool(name="p2", bufs=3) as p2, \
                     tc.tile_pool(name="p2r", bufs=3) as p2r, \
                     tc.tile_pool(name="ps_st", bufs=2, space="PSUM") as ps_st, \
                     tc.tile_pool(name="ps_ot", bufs=1, space="PSUM") as ps_ot, \
                     tc.tile_pool(name="ps_bc", bufs=2, space="PSUM") as ps_bc:
                    # deferred epilogue: PE work (bc matmul) + final multiply
                    # for iteration N are emitted during iteration N+1, after
                    # its first S matmuls, so PE never waits on the reciprocal
                    pending = []

                    def flush_pending():
                        for (po_sb, rc, hp_, q0_, prow_) in pending:
                            bc = ps_bc.tile([64, TCH], F32, tag="bc",
                                            name="bc")
                            nc.tensor.matmul(bc[:], onesbc_r[:], rc[:],
                                             start=True, stop=True)
                            nc.vector.tensor_tensor(
                                attnT[prow_:prow_ + 64, hp_, q0_:q0_ + TCH],
                                po_sb[0:D, :], bc[:], OP.mult)
                        pending.clear()

                    for ch in range(NCH):
                        nkt = 4 * ch + 4   # k-tiles needed (causal)
                        q0 = ch * TCH
                        for hp in range(4):
                            po = [ps_ot.tile([D + 1, TCH], F32, tag=f"ot{i}",
                                             name=f"ot{i}")
                                  for i in range(2)]
                            # S runs one k-tile ahead of AV (software pipeline)
                            ptps = {}
                            offs = {}
                            for kt in range(nkt + 1):
                                if kt < nkt:
                                    j = kt - 4 * ch
                                    off = max(j, 0) * 128
                                    w = TCH - off
                                    offs[kt] = off
                                    stp = ps_st.tile([128, 2 * TCH], F32,
                                                     tag="st", name="stp")
                                    for i in range(2):
                                        prow = i * 64
                                        nc.tensor.matmul(
                                            stp[:, i * TCH:i * TCH + w],
                                            qkT[prow:prow + 64, 4 + hp,
                                                kt * 128:(kt + 1) * 128],
                                            qkT[prow:prow + 64, hp,
                                                q0 + off:q0 + TCH],
                                            start=True, stop=True)
                                    if kt == 1:
                                        flush_pending()
                                    ptp = p2.tile([128, 2 * TCH], F32R,
                                                  tag="pt", name="ptp")
                                    nc.scalar.activation(
                                        ptp[:, 0:TCH + w],
                                        stp[:, 0:TCH + w],
                                        AF.Exp, scale=0.125)
                                    if j >= 0:
                                        ptv = ptp[:].rearrange(
                                            "p (two t) -> p two t", two=2)
                                        nc.vector.tensor_tensor(
                                            ptv[:, :, 0:128], ptv[:, :, 0:128],
                                            mask2[:], OP.mult)
                                    ptps[kt] = ptp
                                if kt > 0:
                                    pk = kt - 1
                                    off = offs.pop(pk)
                                    w = TCH - off
                                    ptp = ptps.pop(pk)
                                    for i in range(2):
                                        nc.tensor.matmul(
                                            po[i][:, off:TCH],
                                            vt[:, pk, 2 * hp + i, :],
                                            ptp[:, i * TCH:i * TCH + w],
                                            start=(pk == 0),
                                            stop=(pk == nkt - 1))
                            for i in range(2):
                                h = 2 * hp + i
                                po_sb = p2r.tile([D + 1, TCH], F32, tag="posb",
                                                 name="posb")
                                nc.scalar.copy(po_sb[:], po[i][:])
                                rc = p2r.tile([1, TCH], F32R, tag="rc")
                                with nc.allow_low_precision(
                                        reason="softmax denom recip"):
                                    nc.vector.reciprocal(rc[:],
                                                         po_sb[D:D + 1, :])
                                pending.append(
                                    (po_sb, rc, hp, q0, (h % 2) * 64))
                    flush_pending()

                # ---------------- phase 3: output projection ----------------
                with tc.tile_pool(name="p3o", bufs=3) as p3o, \
                     tc.tile_pool(name="ps3", bufs=2, space="PSUM") as ps3:
                    for mc in range(8):
                        for tch in range(NCH):
                            pj = ps3.tile([128, TCH], F32, tag="pj")
                            for fc in range(4):
                                nc.tensor.matmul(
                                    pj[:], wp_t[:, fc, mc * 128:(mc + 1) * 128],
                                    attnT[:, fc, tch * TCH:(tch + 1) * TCH],
                                    start=(fc == 0), stop=(fc == 3))
                            ob = p3o.tile([128, TCH], F32, tag="ob")
                            nc.vector.tensor_copy(ob[:], pj[:])
                            nc.sync.dma_start(
                                outT_r[:, mc, tch * TCH:(tch + 1) * TCH], ob[:])
    return nc


# ---------------------------------------------------------------- cached runner
def _make_runner(with_bias=False):
    """Build nc once and return a callable(in_maps) -> list of out dicts,
    with the jitted sharded executable cached across calls."""
    import jax
    from jax.experimental.shard_map import shard_map
    from jax.sharding import Mesh, PartitionSpec

    import concourse.mybir as mybir
    from concourse import bass2jax

    nc = build_nc(with_bias=with_bias)

    partition_name = (nc.partition_id_tensor.name
                      if nc.partition_id_tensor else None)
    in_names = []
    out_names = []
    out_avals = []
    zero_shapes = []
    for alloc in nc.m.functions[0].allocations:
        if not isinstance(alloc, mybir.MemoryLocationSet):
            continue
        name = alloc.memorylocations[0].name
        if alloc.kind == "ExternalInput":
            if name != partition_name:
                in_names.append(name)
        elif alloc.kind == "ExternalOutput":
            out_names.append(name)
            shape = tuple(alloc.tensor_shape)
            dtype = mybir.dt.np(alloc.dtype)
            out_avals.append(jax.core.ShapedArray(shape, dtype))
            zero_shapes.append((shape, dtype))
    n_params = len(in_names)
    all_names = list(in_names + out_names)
    if partition_name is not None:
        all_names.append(partition_name)
    all_names = tuple(all_names)

    def _body(*args):
        operands = list(args)
        if partition_name is not None:
            operands.append(bass2jax.partition_id_tensor())
        outs = bass2jax._bass_exec_p.bind(
            *operands,
            out_avals=tuple(out_avals),
            in_names=all_names,
            out_names=tuple(out_names),
            lowering_input_output_aliases=(),
            sim_require_finite=True,
            sim_require_nnan=True,
            nc=nc,
        )
        return tuple(outs)

    devices = jax.devices()[:8]
    mesh = Mesh(np.asarray(devices), ("core",))
    n_outs = len(out_names)
    sharded = jax.jit(
        shard_map(
            _body, mesh=mesh,
            in_specs=(PartitionSpec("core"),) * (n_params + n_outs),
            out_specs=(PartitionSpec("core"),) * n_outs,
            check_rep=False,
        ),
        donate_argnums=tuple(range(n_params, n_params + n_outs)),
        keep_unused=True,
    )

    def run(in_maps):
        concat_in = [
            np.concatenate([np.asarray(m[name]) for m in in_maps], axis=0)
            for name in in_names
        ]
        concat_zeros = [
            np.zeros((8 * s[0], *s[1:]), dt) for s, dt in zero_shapes
        ]
        out_arrs = sharded(*concat_in, *concat_zeros)
        return [
            {
                name: np.asarray(out_arrs[i]).reshape(8, *zero_shapes[i][0])[c]
                for i, name in enumerate(out_names)
            }
            for c in range(8)
        ]

    return run


# ---------------------------------------------------------------- host wrapper
def make_in_maps(x, w_qkv, b_qkv, w_proj, b_proj):
    in_maps = []
    for b in range(B):
        xT_b = np.ascontiguousarray(x[b].T, dtype=np.float32)
        for hg in range(2):
            s = hg * 512
            wqk = np.ascontiguousarray(
                np.concatenate(
                    [w_qkv[:, s:s + 512], w_qkv[:, 1024 + s:1024 + s + 512]],
                    axis=1), dtype=np.float32)
            wv = np.ascontiguousarray(w_qkv[:, 2048 + s:2048 + s + 512],
                                      dtype=np.float32)
            wp = np.ascontiguousarray(w_proj[s:s + 512, :], dtype=np.float32)
            bqk = np.concatenate(
                [b_qkv[s:s + 512], b_qkv[1024 + s:1024 + s + 512]]
            ).reshape(1, 1024).astype(np.float32)
            bv = b_qkv[2048 + s:2048 + s + 512].reshape(1, 512).astype(np.float32)
            in_maps.append({"xT": xT_b, "w_qk": wqk, "w_v": wv, "w_p": wp,
                            "b_qk": bqk, "b_v": bv})
    return in_maps


def combine(results, b_proj):
    out = np.empty((B, T, C), dtype=np.float32)
    for b in range(B):
        acc = results[2 * b]["outT"] + results[2 * b + 1]["outT"]  # [1024, T]
        out[b] = acc.T
    out += b_proj.astype(np.float32)
    return out


def kernel(x, w_qkv, b_qkv, w_proj, b_proj):
    global _RUNNER
    b_qkv = np.asarray(b_qkv)
    with_bias = bool(np.any(b_qkv))
    if _RUNNER is None or _RUNNER[0] != with_bias:
        _RUNNER = (with_bias, _make_runner(with_bias=with_bias))
    in_maps = make_in_maps(np.asarray(x), np.asarray(w_qkv), b_qkv,
                           np.asarray(w_proj), np.asarray(b_proj))
    results = _RUNNER[1](in_maps)
    return combine(results, np.asarray(b_proj))


# revision 43
# speedup vs baseline: 1.4276x; 1.0093x over previous
"""Causal self-attention (B=4, T=2048, D=1024, H=16, d=64) on 8 TRN2 cores.

Sharding: 8 cores = 4 batches x 2 head-groups (8 heads each). Each core
computes, for its (batch, head-group):
  qk^T = (x @ w_qk)^T           [1024, T]   (q^T rows 0..511, k^T rows 512..1023)
  v    = x @ w_v                [T, 512]    (+ ones column per head -> [.., 65])
  S^T  = K^T.T @ Q^T per head   [k, q] tiles, exp via ACT (scale 1/8 folded)
  P^T causal-masked, AV: oT[65, q] accumulates V'.T @ P^T  (row 64 = denom)
  attn^T = oT[0:64] * (1/denom) (denominator broadcast via K=1 matmul)
  out^T += w_proj_slice.T-chunks @ attn^T  -> [1024, T] partial
Host sums the two head-group partials per batch, adds biases' linear terms,
and transposes back. All matmuls run as float32r (full-rate fp32).
"""

import json

import numpy as np

B = 4
T = 2048
C = 1024
NH = 8          # heads per core
D = 64
TCH = 512       # q/time chunk
NKT = T // 128  # 16 k-tiles
NCH = T // TCH  # 4 chunks

_RUNNER = None


# ---------------------------------------------------------------- BIR legalize
def _legalize_bir_json(bir_bytes):
    """Stock walrus allows only one sem wait per instruction; hoist extras
    onto same-engine NoOps inserted immediately before."""
    bir = json.loads(bir_bytes)
    n = [0]
    changed = False
    for func in bir.get("functions", []):
        for bb in func.get("blocks", []):
            out = []
            for inst in bb.get("instructions", []):
                si = inst.get("sync_info")
                if si:
                    upds = si.get("on_update") or []
                    assert len(upds) <= 1, (inst.get("name"), len(upds))
                waits = (si or {}).get("on_wait") or []
                if len(waits) > 1:
                    changed = True
                    for w in waits[:-1]:
                        n[0] += 1
                        out.append({
                            "debug": inst.get("debug", 0),
                            "engine": inst["engine"],
                            "ins": [],
                            "name": f"I-waitsplit-{n[0]}",
                            "opcode": "NoOp",
                            "outs": [],
                            "sync_info": {"on_update": [], "on_wait": [w]},
                        })
                    si["on_wait"] = [waits[-1]]
                out.append(inst)
            bb["instructions"] = out
    return json.dumps(bir).encode() if changed else bir_bytes


def _install_patch():
    import concourse.bass2jax as b2j
    import concourse.bass_utils as bu

    if getattr(bu, "_waitsplit_patched", False):
        return
    orig = bu.compile_bir_kernel

    def patched(bir_json, tmpdir, neff_name="file.neff"):
        if isinstance(bir_json, str):
            bir_json = bir_json.encode()
        return orig(_legalize_bir_json(bir_json), tmpdir, neff_name=neff_name)

    b2j.compile_bir_kernel = patched
    bu.compile_bir_kernel = patched
    bu._waitsplit_patched = True


# ---------------------------------------------------------------- bass program
def build_nc(with_bias=False):
    _install_patch()
    import concourse.bass as bass
    import concourse.mybir as mybir
    from concourse.tile import TileContext

    F32 = mybir.dt.float32
    F32R = mybir.dt.float32r
    AF = mybir.ActivationFunctionType
    OP = mybir.AluOpType

    nc = bass.Bass("TRN2")
    xT = nc.dram_tensor("xT", [C, T], F32R, kind="ExternalInput")
    w_qk = nc.dram_tensor("w_qk", [C, 1024], F32R, kind="ExternalInput")
    w_v = nc.dram_tensor("w_v", [C, 512], F32R, kind="ExternalInput")
    w_p = nc.dram_tensor("w_p", [512, 1024], F32R, kind="ExternalInput")
    b_qk = nc.dram_tensor("b_qk", [1, 1024], F32R, kind="ExternalInput")
    b_v = nc.dram_tensor("b_v", [1, 512], F32R, kind="ExternalInput")
    outT = nc.dram_tensor("outT", [C, T], F32, kind="ExternalOutput")

    xT_r = xT.rearrange("(co p) t -> p co t", p=128)      # [128, 8, T]
    wqk_r = w_qk.rearrange("(co p) n -> p co n", p=128)   # [128, 8, 1024]
    wv_r = w_v.rearrange("(co p) n -> p co n", p=128)     # [128, 8, 512]
    wp_r = w_p.rearrange("(fo p) n -> p fo n", p=128)     # [128, 4, 1024]
    outT_r = outT.rearrange("(mo p) t -> p mo t", p=128)  # [128, 8, T]

    with TileContext(nc) as tc:
        with tc.tile_pool(name="persist", bufs=1) as pp:
            qkT = pp.tile([128, 8, T], F32R)        # rows: q^T (0..3), k^T (4..7)
            # v' per (k-tile, head): [v (64) | ones (1)] -> AV lhsT [128, 65];
            # PSUM row 64 accumulates the softmax denominator.
            vt = pp.tile([128, NKT, NH, D + 1], F32R)
            onesp_f = pp.tile([128, 1], F32)
            nc.gpsimd.memset(onesp_f[:], 1.0)
            nc.vector.tensor_copy(
                vt[:, :, :, D:D + 1], onesp_f[:].to_broadcast([128, NKT, NH, 1]))
            onesbc_r = pp.tile([1, D], F32R)
            nc.vector.tensor_copy(onesbc_r[:],
                                  onesp_f[0:1, :].to_broadcast([1, D]))
            bqk_t = pp.tile([1, 1024], F32R)
            bv_t = pp.tile([1, 512], F32R)
            nc.sync.dma_start(bqk_t[:], b_qk[:])
            nc.sync.dma_start(bv_t[:], b_v[:])
            if with_bias:
                ones_f = pp.tile([1, TCH], F32)
                nc.gpsimd.memset(ones_f[:], 1.0)
                ones_r = pp.tile([1, TCH], F32R)
                nc.vector.tensor_copy(ones_r[:], ones_f[:])
                onesrow_r = pp.tile([1, 128], F32R)
                nc.vector.tensor_copy(
                    onesrow_r[:], onesp_f[0:1, :].to_broadcast([1, 128]))

            # ---------------- phase 1: qkv projections ----------------
            with tc.tile_pool(name="p1", bufs=1) as p1, \
                 tc.tile_pool(name="p1x", bufs=2) as p1x, \
                 tc.tile_pool(name="ps1", bufs=2, space="PSUM") as ps1, \
                 tc.tile_pool(name="ps1v", bufs=2, space="PSUM") as ps1v:
                wqk_t = p1.tile([128, 8, 1024], F32R)
                wv_t = p1.tile([128, 8, 512], F32R)
                nc.sync.dma_start(wqk_t[:], wqk_r[:])
                nc.sync.dma_start(wv_t[:], wv_r[:])
                for tch in range(NCH):
                    xt = p1x.tile([128, 8, TCH], F32R)
                    nc.sync.dma_start(xt[:], xT_r[:, :, tch * TCH:(tch + 1) * TCH])
                    for mc in range(8):
                        pq = ps1.tile([128, TCH], F32)
                        for cc in range(8):
                            nc.tensor.matmul(
                                pq[:], wqk_t[:, cc, mc * 128:(mc + 1) * 128],
                                xt[:, cc, :], start=(cc == 0),
                                stop=(cc == 7 and not with_bias))
                        if with_bias:
                            nc.tensor.matmul(
                                pq[:], bqk_t[:, mc * 128:(mc + 1) * 128],
                                ones_r[:], start=False, stop=True)
                        nc.vector.tensor_copy(
                            qkT[:, mc, tch * TCH:(tch + 1) * TCH], pq[:])
                    for tt in range(4):
                        pv = ps1v.tile([128, 512], F32)
                        for cc in range(8):
                            nc.tensor.matmul(
                                pv[:], xt[:, cc, tt * 128:(tt + 1) * 128],
                                wv_t[:, cc, :], start=(cc == 0),
                                stop=(cc == 7 and not with_bias))
                        if with_bias:
                            nc.tensor.matmul(
                                pv[:], onesrow_r[:], bv_t[:],
                                start=False, stop=True)
                        nc.vector.tensor_copy(
                            vt[:, tch * 4 + tt, :, 0:D],
                            pv[:].rearrange("p (h d) -> p h d", h=NH))

            # ---------------- phase 2: attention ----------------
            with tc.tile_pool(name="persist2", bufs=1) as pp2:
                attnT = pp2.tile([128, 4, T], F32R)
                # prefetch proj weights so phase 3 starts without a DMA stall
                wp_t = pp2.tile([128, 4, 1024], F32R)
                nc.sync.dma_start(wp_t[:], wp_r[:])
                # single 128x128 lower-tri mask, duplicated for the head pair
                mask2 = pp2.tile([128, 2, 128], F32)
                nc.gpsimd.memset(mask2[:], 1.0)
                for half in range(2):
                    nc.gpsimd.affine_select(
                        out=mask2[:, half, :], in_=mask2[:, half, :],
                        compare_op=OP.is_ge, fill=0.0,
                        base=0, pattern=[[1, 128]], channel_multiplier=-1)

                with tc.tile_pool(name="p2", bufs=4) as p2, \
                     tc.tile_pool(name="p2r", bufs=4) as p2r, \
                     tc.tile_pool(name="ps_st", bufs=2, space="PSUM") as ps_st, \
                     tc.tile_pool(name="ps_ot", bufs=1, space="PSUM") as ps_ot, \
                     tc.tile_pool(name="ps_bc", bufs=2, space="PSUM") as ps_bc:
                    # deferred epilogue: PE work (bc matmul) + final multiply
                    # for iteration N are emitted during iteration N+1, after
                    # its first S matmuls, so PE never waits on the reciprocal
                    pending = []

                    def flush_one():
                        if not pending:
                            return
                        po_sb, rc, hp_, q0_, prow_ = pending.pop(0)
                        bc = ps_bc.tile([64, TCH], F32, tag="bc", name="bc")
                        nc.tensor.matmul(bc[:], onesbc_r[:], rc[:],
                                         start=True, stop=True)
                        nc.vector.tensor_tensor(
                            attnT[prow_:prow_ + 64, hp_, q0_:q0_ + TCH],
                            po_sb[0:D, :], bc[:], OP.mult)

                    def flush_pending():
                        while pending:
                            flush_one()

                    for ch in range(NCH):
                        nkt = 4 * ch + 4   # k-tiles needed (causal)
                        q0 = ch * TCH
                        for hp in range(4):
                            po = [ps_ot.tile([D + 1, TCH], F32, tag=f"ot{i}",
                                             name=f"ot{i}")
                                  for i in range(2)]
                            # S runs two k-tiles ahead of AV (software pipeline)
                            LAG = 2
                            ptps = {}
                            offs = {}
                            for kt in range(nkt + LAG):
                                if kt in (3, 5):
                                    flush_one()
                                if kt < nkt:
                                    j = kt - 4 * ch
                                    off = max(j, 0) * 128
                                    w = TCH - off
                                    offs[kt] = off
                                    stp = ps_st.tile([128, 2 * TCH], F32,
                                                     tag="st", name="stp")
                                    for i in range(2):
                                        prow = i * 64
                                        nc.tensor.matmul(
                                            stp[:, i * TCH:i * TCH + w],
                                            qkT[prow:prow + 64, 4 + hp,
                                                kt * 128:(kt + 1) * 128],
                                            qkT[prow:prow + 64, hp,
                                                q0 + off:q0 + TCH],
                                            start=True, stop=True)
                                    ptp = p2.tile([128, 2 * TCH], F32R,
                                                  tag="pt", name="ptp")
                                    nc.scalar.activation(
                                        ptp[:, 0:TCH + w],
                                        stp[:, 0:TCH + w],
                                        AF.Exp, scale=0.125)
                                    if j >= 0:
                                        ptv = ptp[:].rearrange(
                                            "p (two t) -> p two t", two=2)
                                        nc.vector.tensor_tensor(
                                            ptv[:, :, 0:128], ptv[:, :, 0:128],
                                            mask2[:], OP.mult)
                                    ptps[kt] = ptp
                                if kt >= LAG:
                                    pk = kt - LAG
                                    off = offs.pop(pk)
                                    w = TCH - off
                                    ptp = ptps.pop(pk)
                                    for i in range(2):
                                        nc.tensor.matmul(
                                            po[i][:, off:TCH],
                                            vt[:, pk, 2 * hp + i, :],
                                            ptp[:, i * TCH:i * TCH + w],
                                            start=(pk == 0),
                                            stop=(pk == nkt - 1))
                            for i in range(2):
                                h = 2 * hp + i
                                po_sb = p2r.tile([D + 1, TCH], F32, tag="posb",
                                                 name="posb")
                                nc.scalar.copy(po_sb[:], po[i][:])
                                rc = p2r.tile([1, TCH], F32R, tag="rc")
                                with nc.allow_low_precision(
                                        reason="softmax denom recip"):
                                    nc.vector.reciprocal(rc[:],
                                                         po_sb[D:D + 1, :])
                                pending.append(
                                    (po_sb, rc, hp, q0, (h % 2) * 64))
                    flush_pending()

                # ---------------- phase 3: output projection ----------------
                with tc.tile_pool(name="p3o", bufs=3) as p3o, \
                     tc.tile_pool(name="ps3", bufs=2, space="PSUM") as ps3:
                    for mc in range(8):
                        for tch in range(NCH):
                            pj = ps3.tile([128, TCH], F32, tag="pj")
                            for fc in range(4):
                                nc.tensor.matmul(
                                    pj[:], wp_t[:, fc, mc * 128:(mc + 1) * 128],
                                    attnT[:, fc, tch * TCH:(tch + 1) * TCH],
                                    start=(fc == 0), stop=(fc == 3))
                            ob = p3o.tile([128, TCH], F32, tag="ob")
                            nc.vector.tensor_copy(ob[:], pj[:])
                            nc.sync.dma_start(
                                outT_r[:, mc, tch * TCH:(tch + 1) * TCH], ob[:])
    return nc


# ---------------------------------------------------------------- cached runner
def _make_runner(with_bias=False):
    """Build nc once and return a callable(in_maps) -> list of out dicts,
    with the jitted sharded executable cached across calls."""
    import jax
    from jax.experimental.shard_map import shard_map
    from jax.sharding import Mesh, PartitionSpec

    import concourse.mybir as mybir
    from concourse import bass2jax

    nc = build_nc(with_bias=with_bias)

    partition_name = (nc.partition_id_tensor.name
                      if nc.partition_id_tensor else None)
    in_names = []
    out_names = []
    out_avals = []
    zero_shapes = []
    for alloc in nc.m.functions[0].allocations:
        if not isinstance(alloc, mybir.MemoryLocationSet):
            continue
        name = alloc.memorylocations[0].name
        if alloc.kind == "ExternalInput":
            if name != partition_name:
                in_names.append(name)
        elif alloc.kind == "ExternalOutput":
            out_names.append(name)
            shape = tuple(alloc.tensor_shape)
            dtype = mybir.dt.np(alloc.dtype)
            out_avals.append(jax.core.ShapedArray(shape, dtype))
            zero_shapes.append((shape, dtype))
    n_params = len(in_names)
    all_names = list(in_names + out_names)
    if partition_name is not None:
        all_names.append(partition_name)
    all_names = tuple(all_names)

    def _body(*args):
        operands = list(args)
        if partition_name is not None:
            operands.append(bass2jax.partition_id_tensor())
        outs = bass2jax._bass_exec_p.bind(
            *operands,
            out_avals=tuple(out_avals),
            in_names=all_names,
            out_names=tuple(out_names),
            lowering_input_output_aliases=(),
            sim_require_finite=True,
            sim_require_nnan=True,
            nc=nc,
        )
        return tuple(outs)

    devices = jax.devices()[:8]
    mesh = Mesh(np.asarray(devices), ("core",))
    n_outs = len(out_names)
    sharded = jax.jit(
        shard_map(
            _body, mesh=mesh,
            in_specs=(PartitionSpec("core"),) * (n_params + n_outs),
            out_specs=(PartitionSpec("core"),) * n_outs,
            check_rep=False,
        ),
        donate_argnums=tuple(range(n_params, n_params + n_outs)),
        keep_unused=True,
    )

    def run(in_maps):
        concat_in = [
            np.concatenate([np.asarray(m[name]) for m in in_maps], axis=0)
            for name in in_names
        ]
        concat_zeros = [
            np.zeros((8 * s[0], *s[1:]), dt) for s, dt in zero_shapes
        ]
        out_arrs = sharded(*concat_in, *concat_zeros)
        return [
            {
                name: np.asarray(out_arrs[i]).reshape(8, *zero_shapes[i][0])[c]
                for i, name in enumerate(out_names)
            }
            for c in range(8)
        ]

    return run


# ---------------------------------------------------------------- host wrapper
def make_in_maps(x, w_qkv, b_qkv, w_proj, b_proj):
    in_maps = []
    for b in range(B):
        xT_b = np.ascontiguousarray(x[b].T, dtype=np.float32)
        for hg in range(2):
            s = hg * 512
            wqk = np.ascontiguousarray(
                np.concatenate(
                    [w_qkv[:, s:s + 512], w_qkv[:, 1024 + s:1024 + s + 512]],
                    axis=1), dtype=np.float32)
            wv = np.ascontiguousarray(w_qkv[:, 2048 + s:2048 + s + 512],
                                      dtype=np.float32)
            wp = np.ascontiguousarray(w_proj[s:s + 512, :], dtype=np.float32)
            bqk = np.concatenate(
                [b_qkv[s:s + 512], b_qkv[1024 + s:1024 + s + 512]]
            ).reshape(1, 1024).astype(np.float32)
            bv = b_qkv[2048 + s:2048 + s + 512].reshape(1, 512).astype(np.float32)
            in_maps.append({"xT": xT_b, "w_qk": wqk, "w_v": wv, "w_p": wp,
                            "b_qk": bqk, "b_v": bv})
    return in_maps


def combine(results, b_proj):
    out = np.empty((B, T, C), dtype=np.float32)
    for b in range(B):
        acc = results[2 * b]["outT"] + results[2 * b + 1]["outT"]  # [1024, T]
        out[b] = acc.T
    out += b_proj.astype(np.float32)
    return out


def kernel(x, w_qkv, b_qkv, w_proj, b_proj):
    global _RUNNER
    b_qkv = np.asarray(b_qkv)
    with_bias = bool(np.any(b_qkv))
    if _RUNNER is None or _RUNNER[0] != with_bias:
        _RUNNER = (with_bias, _make_runner(with_bias=with_bias))
    in_maps = make_in_maps(np.asarray(x), np.asarray(w_qkv), b_qkv,
                           np.asarray(w_proj), np.asarray(b_proj))
    results = _RUNNER[1](in_maps)
    return combine(results, np.asarray(b_proj))


# revision 44
# speedup vs baseline: 1.5456x; 1.0827x over previous
"""Causal self-attention (B=4, T=2048, D=1024, H=16, d=64) on 8 TRN2 cores.

Sharding: 8 cores = 4 batches x 2 head-groups (8 heads each). Each core
computes, for its (batch, head-group):
  qk^T = (x @ w_qk)^T           [1024, T]   (q^T rows 0..511, k^T rows 512..1023)
  v    = x @ w_v                [T, 512]    (+ ones column per head -> [.., 65])
  S^T  = K^T.T @ Q^T per head   [k, q] tiles, exp via ACT (scale 1/8 folded)
  P^T causal-masked, AV: oT[65, q] accumulates V'.T @ P^T  (row 64 = denom)
  attn^T = oT[0:64] * (1/denom) (denominator broadcast via K=1 matmul)
  out^T += w_proj_slice.T-chunks @ attn^T  -> [1024, T] partial
Host sums the two head-group partials per batch, adds biases' linear terms,
and transposes back. All matmuls run as float32r (full-rate fp32).
"""

import json

import numpy as np

B = 4
T = 2048
C = 1024
NH = 8          # heads per core
D = 64
TCH = 512       # q/time chunk
NKT = T // 128  # 16 k-tiles
NCH = T // TCH  # 4 chunks

_RUNNER = None


# ---------------------------------------------------------------- BIR legalize
def _legalize_bir_json(bir_bytes):
    """Stock walrus allows only one sem wait per instruction; hoist extras
    onto same-engine NoOps inserted immediately before."""
    bir = json.loads(bir_bytes)
    n = [0]
    changed = False
    for func in bir.get("functions", []):
        for bb in func.get("blocks", []):
            out = []
            for inst in bb.get("instructions", []):
                si = inst.get("sync_info")
                if si:
                    upds = si.get("on_update") or []
                    assert len(upds) <= 1, (inst.get("name"), len(upds))
                waits = (si or {}).get("on_wait") or []
                if len(waits) > 1:
                    changed = True
                    for w in waits[:-1]:
                        n[0] += 1
                        out.append({
                            "debug": inst.get("debug", 0),
                            "engine": inst["engine"],
                            "ins": [],
                            "name": f"I-waitsplit-{n[0]}",
                            "opcode": "NoOp",
                            "outs": [],
                            "sync_info": {"on_update": [], "on_wait": [w]},
                        })
                    si["on_wait"] = [waits[-1]]
                out.append(inst)
            bb["instructions"] = out
    return json.dumps(bir).encode() if changed else bir_bytes


def _install_patch():
    import concourse.bass2jax as b2j
    import concourse.bass_utils as bu

    if getattr(bu, "_waitsplit_patched", False):
        return
    orig = bu.compile_bir_kernel

    def patched(bir_json, tmpdir, neff_name="file.neff"):
        if isinstance(bir_json, str):
            bir_json = bir_json.encode()
        return orig(_legalize_bir_json(bir_json), tmpdir, neff_name=neff_name)

    b2j.compile_bir_kernel = patched
    bu.compile_bir_kernel = patched
    bu._waitsplit_patched = True


# ---------------------------------------------------------------- bass program
def build_nc(with_bias=False):
    _install_patch()
    import concourse.bass as bass
    import concourse.mybir as mybir
    from concourse.tile import TileContext

    F32 = mybir.dt.float32
    F32R = mybir.dt.float32r
    AF = mybir.ActivationFunctionType
    OP = mybir.AluOpType

    nc = bass.Bass("TRN2")
    xT = nc.dram_tensor("xT", [C, T], F32R, kind="ExternalInput")
    w_qk = nc.dram_tensor("w_qk", [C, 1024], F32R, kind="ExternalInput")
    w_v = nc.dram_tensor("w_v", [C, 512], F32R, kind="ExternalInput")
    w_p = nc.dram_tensor("w_p", [512, 1024], F32R, kind="ExternalInput")
    b_qk = nc.dram_tensor("b_qk", [1, 1024], F32R, kind="ExternalInput")
    b_v = nc.dram_tensor("b_v", [1, 512], F32R, kind="ExternalInput")
    outT = nc.dram_tensor("outT", [C, T], F32, kind="ExternalOutput")

    xT_r = xT.rearrange("(co p) t -> p co t", p=128)      # [128, 8, T]
    wqk_r = w_qk.rearrange("(co p) n -> p co n", p=128)   # [128, 8, 1024]
    wv_r = w_v.rearrange("(co p) n -> p co n", p=128)     # [128, 8, 512]
    wp_r = w_p.rearrange("(fo p) n -> p fo n", p=128)     # [128, 4, 1024]
    outT_r = outT.rearrange("(mo p) t -> p mo t", p=128)  # [128, 8, T]

    with TileContext(nc) as tc:
        with tc.tile_pool(name="persist", bufs=1) as pp:
            qkT = pp.tile([128, 8, T], F32R)        # rows: q^T (0..3), k^T (4..7)
            # v' per (k-tile, head): [v (64) | ones (1)] -> AV lhsT [128, 65];
            # PSUM row 64 accumulates the softmax denominator.
            vt = pp.tile([128, NKT, NH, D + 1], F32R)
            onesp_f = pp.tile([128, 1], F32)
            nc.gpsimd.memset(onesp_f[:], 1.0)
            nc.vector.tensor_copy(
                vt[:, :, :, D:D + 1], onesp_f[:].to_broadcast([128, NKT, NH, 1]))
            onesbc_r = pp.tile([1, D], F32R)
            nc.vector.tensor_copy(onesbc_r[:],
                                  onesp_f[0:1, :].to_broadcast([1, D]))
            bqk_t = pp.tile([1, 1024], F32R)
            bv_t = pp.tile([1, 512], F32R)
            nc.sync.dma_start(bqk_t[:], b_qk[:])
            nc.sync.dma_start(bv_t[:], b_v[:])
            if with_bias:
                ones_f = pp.tile([1, TCH], F32)
                nc.gpsimd.memset(ones_f[:], 1.0)
                ones_r = pp.tile([1, TCH], F32R)
                nc.vector.tensor_copy(ones_r[:], ones_f[:])
                onesrow_r = pp.tile([1, 128], F32R)
                nc.vector.tensor_copy(
                    onesrow_r[:], onesp_f[0:1, :].to_broadcast([1, 128]))

            # ---------------- phase 1: qkv projections ----------------
            with tc.tile_pool(name="p1", bufs=1) as p1, \
                 tc.tile_pool(name="p1x", bufs=2) as p1x, \
                 tc.tile_pool(name="ps1", bufs=2, space="PSUM") as ps1, \
                 tc.tile_pool(name="ps1v", bufs=2, space="PSUM") as ps1v:
                wqk_t = p1.tile([128, 8, 1024], F32R)
                wv_t = p1.tile([128, 8, 512], F32R)
                nc.sync.dma_start(wqk_t[:], wqk_r[:])
                nc.sync.dma_start(wv_t[:], wv_r[:])
                for tch in range(NCH):
                    xt = p1x.tile([128, 8, TCH], F32R)
                    nc.sync.dma_start(xt[:], xT_r[:, :, tch * TCH:(tch + 1) * TCH])
                    for mc in range(8):
                        pq = ps1.tile([128, TCH], F32)
                        for cc in range(8):
                            nc.tensor.matmul(
                                pq[:], wqk_t[:, cc, mc * 128:(mc + 1) * 128],
                                xt[:, cc, :], start=(cc == 0),
                                stop=(cc == 7 and not with_bias))
                        if with_bias:
                            nc.tensor.matmul(
                                pq[:], bqk_t[:, mc * 128:(mc + 1) * 128],
                                ones_r[:], start=False, stop=True)
                        nc.vector.tensor_copy(
                            qkT[:, mc, tch * TCH:(tch + 1) * TCH], pq[:])
                    for tt in range(4):
                        pv = ps1v.tile([128, 512], F32)
                        for cc in range(8):
                            nc.tensor.matmul(
                                pv[:], xt[:, cc, tt * 128:(tt + 1) * 128],
                                wv_t[:, cc, :], start=(cc == 0),
                                stop=(cc == 7 and not with_bias))
                        if with_bias:
                            nc.tensor.matmul(
                                pv[:], onesrow_r[:], bv_t[:],
                                start=False, stop=True)
                        nc.vector.tensor_copy(
                            vt[:, tch * 4 + tt, :, 0:D],
                            pv[:].rearrange("p (h d) -> p h d", h=NH))

            # ---------------- phase 2: attention ----------------
            with tc.tile_pool(name="persist2", bufs=1) as pp2:
                attnT = pp2.tile([128, 4, T], F32R)
                # prefetch proj weights so phase 3 starts without a DMA stall
                wp_t = pp2.tile([128, 4, 1024], F32R)
                nc.sync.dma_start(wp_t[:], wp_r[:])
                # single 128x128 lower-tri mask, duplicated for the head pair
                mask2 = pp2.tile([128, 2, 128], F32)
                nc.gpsimd.memset(mask2[:], 1.0)
                for half in range(2):
                    nc.gpsimd.affine_select(
                        out=mask2[:, half, :], in_=mask2[:, half, :],
                        compare_op=OP.is_ge, fill=0.0,
                        base=0, pattern=[[1, 128]], channel_multiplier=-1)

                with tc.tile_pool(name="p2", bufs=6) as p2, \
                     tc.tile_pool(name="p2r", bufs=4) as p2r, \
                     tc.tile_pool(name="ps_st", bufs=3, space="PSUM") as ps_st, \
                     tc.tile_pool(name="ps_ot", bufs=2, space="PSUM") as ps_ot:
                    # deferred epilogue: PE work (bc matmul) + final multiply
                    # for iteration N are emitted during iteration N+1, after
                    # its first S matmuls, so PE never waits on the reciprocal
                    pending = []

                    def flush_one():
                        if not pending:
                            return
                        po_sb, rc, hp_, q0_, prow_ = pending.pop(0)
                        bc = ps_ot.tile([64, TCH], F32, tag="ot", name="bc")
                        nc.tensor.matmul(bc[:], onesbc_r[:], rc[:],
                                         start=True, stop=True)
                        nc.vector.tensor_tensor(
                            attnT[prow_:prow_ + 64, hp_, q0_:q0_ + TCH],
                            po_sb[0:D, :], bc[:], OP.mult)

                    def flush_pending():
                        while pending:
                            flush_one()

                    for ch in range(NCH):
                        nkt = 4 * ch + 4   # k-tiles needed (causal)
                        q0 = ch * TCH
                        for hp in range(4):
                            po = [ps_ot.tile([D + 1, TCH], F32, tag="ot",
                                             name=f"ot{i}")
                                  for i in range(2)]
                            # S runs two k-tiles ahead of AV (software pipeline)
                            LAG = 3
                            ptps = {}
                            offs = {}
                            for kt in range(nkt + LAG):
                                if kt in (4, 6):
                                    flush_one()
                                if kt < nkt:
                                    j = kt - 4 * ch
                                    off = max(j, 0) * 128
                                    w = TCH - off
                                    offs[kt] = off
                                    stp = ps_st.tile([128, 2 * TCH], F32,
                                                     tag="st", name="stp")
                                    for i in range(2):
                                        prow = i * 64
                                        nc.tensor.matmul(
                                            stp[:, i * TCH:i * TCH + w],
                                            qkT[prow:prow + 64, 4 + hp,
                                                kt * 128:(kt + 1) * 128],
                                            qkT[prow:prow + 64, hp,
                                                q0 + off:q0 + TCH],
                                            start=True, stop=True)
                                    ptp = p2.tile([128, 2 * TCH], F32R,
                                                  tag="pt", name="ptp")
                                    nc.scalar.activation(
                                        ptp[:, 0:TCH + w],
                                        stp[:, 0:TCH + w],
                                        AF.Exp, scale=0.125)
                                    if j >= 0:
                                        ptv = ptp[:].rearrange(
                                            "p (two t) -> p two t", two=2)
                                        nc.vector.tensor_tensor(
                                            ptv[:, :, 0:128], ptv[:, :, 0:128],
                                            mask2[:], OP.mult)
                                    ptps[kt] = ptp
                                if kt >= LAG:
                                    pk = kt - LAG
                                    off = offs.pop(pk)
                                    w = TCH - off
                                    ptp = ptps.pop(pk)
                                    for i in range(2):
                                        nc.tensor.matmul(
                                            po[i][:, off:TCH],
                                            vt[:, pk, 2 * hp + i, :],
                                            ptp[:, i * TCH:i * TCH + w],
                                            start=(pk == 0),
                                            stop=(pk == nkt - 1))
                            for i in range(2):
                                h = 2 * hp + i
                                po_sb = p2r.tile([D + 1, TCH], F32, tag="posb",
                                                 name="posb")
                                nc.scalar.copy(po_sb[:], po[i][:])
                                rc = p2r.tile([1, TCH], F32R, tag="rc")
                                with nc.allow_low_precision(
                                        reason="softmax denom recip"):
                                    nc.vector.reciprocal(rc[:],
                                                         po_sb[D:D + 1, :])
                                pending.append(
                                    (po_sb, rc, hp, q0, (h % 2) * 64))
                    flush_pending()

                # ---------------- phase 3: output projection ----------------
                with tc.tile_pool(name="p3o", bufs=3) as p3o, \
                     tc.tile_pool(name="ps3", bufs=2, space="PSUM") as ps3:
                    for mc in range(8):
                        for tch in range(NCH):
                            pj = ps3.tile([128, TCH], F32, tag="pj")
                            for fc in range(4):
                                nc.tensor.matmul(
                                    pj[:], wp_t[:, fc, mc * 128:(mc + 1) * 128],
                                    attnT[:, fc, tch * TCH:(tch + 1) * TCH],
                                    start=(fc == 0), stop=(fc == 3))
                            ob = p3o.tile([128, TCH], F32, tag="ob")
                            nc.vector.tensor_copy(ob[:], pj[:])
                            nc.sync.dma_start(
                                outT_r[:, mc, tch * TCH:(tch + 1) * TCH], ob[:])
    return nc


# ---------------------------------------------------------------- cached runner
def _make_runner(with_bias=False):
    """Build nc once and return a callable(in_maps) -> list of out dicts,
    with the jitted sharded executable cached across calls."""
    import jax
    from jax.experimental.shard_map import shard_map
    from jax.sharding import Mesh, PartitionSpec

    import concourse.mybir as mybir
    from concourse import bass2jax

    nc = build_nc(with_bias=with_bias)

    partition_name = (nc.partition_id_tensor.name
                      if nc.partition_id_tensor else None)
    in_names = []
    out_names = []
    out_avals = []
    zero_shapes = []
    for alloc in nc.m.functions[0].allocations:
        if not isinstance(alloc, mybir.MemoryLocationSet):
            continue
        name = alloc.memorylocations[0].name
        if alloc.kind == "ExternalInput":
            if name != partition_name:
                in_names.append(name)
        elif alloc.kind == "ExternalOutput":
            out_names.append(name)
            shape = tuple(alloc.tensor_shape)
            dtype = mybir.dt.np(alloc.dtype)
            out_avals.append(jax.core.ShapedArray(shape, dtype))
            zero_shapes.append((shape, dtype))
    n_params = len(in_names)
    all_names = list(in_names + out_names)
    if partition_name is not None:
        all_names.append(partition_name)
    all_names = tuple(all_names)

    def _body(*args):
        operands = list(args)
        if partition_name is not None:
            operands.append(bass2jax.partition_id_tensor())
        outs = bass2jax._bass_exec_p.bind(
            *operands,
            out_avals=tuple(out_avals),
            in_names=all_names,
            out_names=tuple(out_names),
            lowering_input_output_aliases=(),
            sim_require_finite=True,
            sim_require_nnan=True,
            nc=nc,
        )
        return tuple(outs)

    devices = jax.devices()[:8]
    mesh = Mesh(np.asarray(devices), ("core",))
    n_outs = len(out_names)
    sharded = jax.jit(
        shard_map(
            _body, mesh=mesh,
            in_specs=(PartitionSpec("core"),) * (n_params + n_outs),
            out_specs=(PartitionSpec("core"),) * n_outs,
            check_rep=False,
        ),
        donate_argnums=tuple(range(n_params, n_params + n_outs)),
        keep_unused=True,
    )

    def run(in_maps):
        concat_in = [
            np.concatenate([np.asarray(m[name]) for m in in_maps], axis=0)
            for name in in_names
        ]
        concat_zeros = [
            np.zeros((8 * s[0], *s[1:]), dt) for s, dt in zero_shapes
        ]
        out_arrs = sharded(*concat_in, *concat_zeros)
        return [
            {
                name: np.asarray(out_arrs[i]).reshape(8, *zero_shapes[i][0])[c]
                for i, name in enumerate(out_names)
            }
            for c in range(8)
        ]

    return run


# ---------------------------------------------------------------- host wrapper
def make_in_maps(x, w_qkv, b_qkv, w_proj, b_proj):
    in_maps = []
    for b in range(B):
        xT_b = np.ascontiguousarray(x[b].T, dtype=np.float32)
        for hg in range(2):
            s = hg * 512
            wqk = np.ascontiguousarray(
                np.concatenate(
                    [w_qkv[:, s:s + 512], w_qkv[:, 1024 + s:1024 + s + 512]],
                    axis=1), dtype=np.float32)
            wv = np.ascontiguousarray(w_qkv[:, 2048 + s:2048 + s + 512],
                                      dtype=np.float32)
            wp = np.ascontiguousarray(w_proj[s:s + 512, :], dtype=np.float32)
            bqk = np.concatenate(
                [b_qkv[s:s + 512], b_qkv[1024 + s:1024 + s + 512]]
            ).reshape(1, 1024).astype(np.float32)
            bv = b_qkv[2048 + s:2048 + s + 512].reshape(1, 512).astype(np.float32)
            in_maps.append({"xT": xT_b, "w_qk": wqk, "w_v": wv, "w_p": wp,
                            "b_qk": bqk, "b_v": bv})
    return in_maps


def combine(results, b_proj):
    out = np.empty((B, T, C), dtype=np.float32)
    for b in range(B):
        acc = results[2 * b]["outT"] + results[2 * b + 1]["outT"]  # [1024, T]
        out[b] = acc.T
    out += b_proj.astype(np.float32)
    return out


def kernel(x, w_qkv, b_qkv, w_proj, b_proj):
    global _RUNNER
    b_qkv = np.asarray(b_qkv)
    with_bias = bool(np.any(b_qkv))
    if _RUNNER is None or _RUNNER[0] != with_bias:
        _RUNNER = (with_bias, _make_runner(with_bias=with_bias))
    in_maps = make_in_maps(np.asarray(x), np.asarray(w_qkv), b_qkv,
                           np.asarray(w_proj), np.asarray(b_proj))
    results = _RUNNER[1](in_maps)
    return combine(results, np.asarray(b_proj))


# revision 45
# speedup vs baseline: 1.6490x; 1.0669x over previous
"""Causal self-attention (B=4, T=2048, D=1024, H=16, d=64) on 8 TRN2 cores.

Sharding: 8 cores = 4 batches x 2 head-groups (8 heads each). Each core
computes, for its (batch, head-group):
  qk^T = (x @ w_qk)^T           [1024, T]   (q^T rows 0..511, k^T rows 512..1023)
  v    = x @ w_v                [T, 512]    (+ ones column per head -> [.., 65])
  S^T  = K^T.T @ Q^T per head   [k, q] tiles, exp via ACT (scale 1/8 folded)
  P^T causal-masked, AV: oT[65, q] accumulates V'.T @ P^T  (row 64 = denom)
  attn^T = oT[0:64] * (1/denom) (denominator broadcast via K=1 matmul)
  out^T += w_proj_slice.T-chunks @ attn^T  -> [1024, T] partial
Host sums the two head-group partials per batch, adds biases' linear terms,
and transposes back. All matmuls run as float32r (full-rate fp32).
"""

import json

import numpy as np

B = 4
T = 2048
C = 1024
NH = 8          # heads per core
D = 64
TCH = 512       # q/time chunk
NKT = T // 128  # 16 k-tiles
NCH = T // TCH  # 4 chunks

_RUNNER = None


# ---------------------------------------------------------------- BIR legalize
def _legalize_bir_json(bir_bytes):
    """Stock walrus allows only one sem wait per instruction; hoist extras
    onto same-engine NoOps inserted immediately before."""
    bir = json.loads(bir_bytes)
    n = [0]
    changed = False
    for func in bir.get("functions", []):
        for bb in func.get("blocks", []):
            out = []
            for inst in bb.get("instructions", []):
                si = inst.get("sync_info")
                if si:
                    upds = si.get("on_update") or []
                    assert len(upds) <= 1, (inst.get("name"), len(upds))
                waits = (si or {}).get("on_wait") or []
                if len(waits) > 1:
                    changed = True
                    for w in waits[:-1]:
                        n[0] += 1
                        out.append({
                            "debug": inst.get("debug", 0),
                            "engine": inst["engine"],
                            "ins": [],
                            "name": f"I-waitsplit-{n[0]}",
                            "opcode": "NoOp",
                            "outs": [],
                            "sync_info": {"on_update": [], "on_wait": [w]},
                        })
                    si["on_wait"] = [waits[-1]]
                out.append(inst)
            bb["instructions"] = out
    return json.dumps(bir).encode() if changed else bir_bytes


def _install_patch():
    import concourse.bass2jax as b2j
    import concourse.bass_utils as bu

    if getattr(bu, "_waitsplit_patched", False):
        return
    orig = bu.compile_bir_kernel

    def patched(bir_json, tmpdir, neff_name="file.neff"):
        if isinstance(bir_json, str):
            bir_json = bir_json.encode()
        return orig(_legalize_bir_json(bir_json), tmpdir, neff_name=neff_name)

    b2j.compile_bir_kernel = patched
    bu.compile_bir_kernel = patched
    bu._waitsplit_patched = True


# ---------------------------------------------------------------- bass program
def build_nc(with_bias=False):
    _install_patch()
    import concourse.bass as bass
    import concourse.mybir as mybir
    from concourse.tile import TileContext

    F32 = mybir.dt.float32
    F32R = mybir.dt.float32r
    AF = mybir.ActivationFunctionType
    OP = mybir.AluOpType

    nc = bass.Bass("TRN2")
    xT = nc.dram_tensor("xT", [C, T], F32R, kind="ExternalInput")
    w_qk = nc.dram_tensor("w_qk", [C, 1024], F32R, kind="ExternalInput")
    w_v = nc.dram_tensor("w_v", [C, 512], F32R, kind="ExternalInput")
    w_p = nc.dram_tensor("w_p", [512, 1024], F32R, kind="ExternalInput")
    b_qk = nc.dram_tensor("b_qk", [1, 1024], F32R, kind="ExternalInput")
    b_v = nc.dram_tensor("b_v", [1, 512], F32R, kind="ExternalInput")
    outT = nc.dram_tensor("outT", [C, T], F32, kind="ExternalOutput")

    xT_r = xT.rearrange("(co p) t -> p co t", p=128)      # [128, 8, T]
    wqk_r = w_qk.rearrange("(co p) n -> p co n", p=128)   # [128, 8, 1024]
    wv_r = w_v.rearrange("(co p) n -> p co n", p=128)     # [128, 8, 512]
    wp_r = w_p.rearrange("(fo p) n -> p fo n", p=128)     # [128, 4, 1024]
    outT_r = outT.rearrange("(mo p) t -> p mo t", p=128)  # [128, 8, T]

    with TileContext(nc) as tc:
        with tc.tile_pool(name="persist", bufs=1) as pp:
            qkT = pp.tile([128, 8, T], F32R)        # rows: q^T (0..3), k^T (4..7)
            # v' per (k-tile, head): [v (64) | ones (1)] -> AV lhsT [128, 65];
            # PSUM row 64 accumulates the softmax denominator.
            vt = pp.tile([128, NKT, NH, D + 1], F32R)
            onesp_f = pp.tile([128, 1], F32)
            nc.gpsimd.memset(onesp_f[:], 1.0)
            nc.vector.tensor_copy(
                vt[:, :, :, D:D + 1], onesp_f[:].to_broadcast([128, NKT, NH, 1]))
            onesbc_r = pp.tile([1, D], F32R)
            nc.vector.tensor_copy(onesbc_r[:],
                                  onesp_f[0:1, :].to_broadcast([1, D]))
            bqk_t = pp.tile([1, 1024], F32R)
            bv_t = pp.tile([1, 512], F32R)
            nc.sync.dma_start(bqk_t[:], b_qk[:])
            nc.sync.dma_start(bv_t[:], b_v[:])
            if with_bias:
                ones_f = pp.tile([1, TCH], F32)
                nc.gpsimd.memset(ones_f[:], 1.0)
                ones_r = pp.tile([1, TCH], F32R)
                nc.vector.tensor_copy(ones_r[:], ones_f[:])
                onesrow_r = pp.tile([1, 128], F32R)
                nc.vector.tensor_copy(
                    onesrow_r[:], onesp_f[0:1, :].to_broadcast([1, 128]))

            # ---------------- phase 1: qkv projections ----------------
            with tc.tile_pool(name="p1", bufs=1) as p1, \
                 tc.tile_pool(name="p1x", bufs=2) as p1x, \
                 tc.tile_pool(name="ps1", bufs=2, space="PSUM") as ps1, \
                 tc.tile_pool(name="ps1v", bufs=2, space="PSUM") as ps1v:
                wqk_t = p1.tile([128, 8, 1024], F32R)
                wv_t = p1.tile([128, 8, 512], F32R)
                nc.sync.dma_start(wqk_t[:], wqk_r[:])
                nc.sync.dma_start(wv_t[:], wv_r[:])
                for tch in range(NCH):
                    xt = p1x.tile([128, 8, TCH], F32R)
                    nc.sync.dma_start(xt[:], xT_r[:, :, tch * TCH:(tch + 1) * TCH])
                    for mc in range(8):
                        pq = ps1.tile([128, TCH], F32)
                        for cc in range(8):
                            nc.tensor.matmul(
                                pq[:], wqk_t[:, cc, mc * 128:(mc + 1) * 128],
                                xt[:, cc, :], start=(cc == 0),
                                stop=(cc == 7 and not with_bias))
                        if with_bias:
                            nc.tensor.matmul(
                                pq[:], bqk_t[:, mc * 128:(mc + 1) * 128],
                                ones_r[:], start=False, stop=True)
                        nc.vector.tensor_copy(
                            qkT[:, mc, tch * TCH:(tch + 1) * TCH], pq[:])
                    for tt in range(4):
                        pv = ps1v.tile([128, 512], F32)
                        for cc in range(8):
                            nc.tensor.matmul(
                                pv[:], xt[:, cc, tt * 128:(tt + 1) * 128],
                                wv_t[:, cc, :], start=(cc == 0),
                                stop=(cc == 7 and not with_bias))
                        if with_bias:
                            nc.tensor.matmul(
                                pv[:], onesrow_r[:], bv_t[:],
                                start=False, stop=True)
                        nc.vector.tensor_copy(
                            vt[:, tch * 4 + tt, :, 0:D],
                            pv[:].rearrange("p (h d) -> p h d", h=NH))

            # ---------------- phase 2: attention ----------------
            with tc.tile_pool(name="persist2", bufs=1) as pp2:
                attnT = pp2.tile([128, 4, T], F32R)
                # prefetch proj weights so phase 3 starts without a DMA stall
                wp_t = pp2.tile([128, 4, 1024], F32R)
                nc.sync.dma_start(wp_t[:], wp_r[:])
                # single 128x128 lower-tri mask, duplicated for the head pair
                mask2 = pp2.tile([128, 2, 128], F32)
                nc.gpsimd.memset(mask2[:], 1.0)
                for half in range(2):
                    nc.gpsimd.affine_select(
                        out=mask2[:, half, :], in_=mask2[:, half, :],
                        compare_op=OP.is_ge, fill=0.0,
                        base=0, pattern=[[1, 128]], channel_multiplier=-1)
                # per-pass denominator tiles: 4 heads' denominators parked at
                # partitions 0/32/64/96, one batched reciprocal per pass
                passes = [(ch, hpp) for ch in range(NCH) for hpp in range(2)]
                rcps = []
                for p in range(len(passes)):
                    r_ = pp2.tile([128, TCH], F32R, tag=f"rcp{p}",
                                  name=f"rcp{p}")
                    rcps.append(r_)
                onesall_r = pp2.tile([128, D], F32R)
                nc.vector.tensor_copy(onesall_r[:],
                                      onesp_f[:].to_broadcast([128, D]))

                LAG = 2
                with tc.tile_pool(name="p2", bufs=5) as p2, \
                     tc.tile_pool(name="ps_st", bufs=1, space="PSUM") as ps_st, \
                     tc.tile_pool(name="ps_ot", bufs=4, space="PSUM") as ps_ot:
                    for pidx, (ch, hpp) in enumerate(passes):
                        nkt = 4 * ch + 4   # k-tiles needed (causal)
                        q0 = ch * TCH
                        hps = (2 * hpp, 2 * hpp + 1)
                        po = {}
                        for hp in hps:
                            po[hp] = [ps_ot.tile([D + 1, TCH], F32, tag="ot",
                                                 name=f"po{hp}_{i}")
                                      for i in range(2)]
                        ptps = {}
                        offs = {}
                        for kt in range(nkt + LAG):
                            for x, hp in enumerate(hps):
                                if kt < nkt:
                                    j = kt - 4 * ch
                                    off = max(j, 0) * 128
                                    w = TCH - off
                                    offs[(x, kt)] = off
                                    stp = ps_st.tile([128, 2 * TCH], F32,
                                                     tag=f"st{x}",
                                                     name=f"st{x}")
                                    for i in range(2):
                                        prow = i * 64
                                        nc.tensor.matmul(
                                            stp[:, i * TCH:i * TCH + w],
                                            qkT[prow:prow + 64, 4 + hp,
                                                kt * 128:(kt + 1) * 128],
                                            qkT[prow:prow + 64, hp,
                                                q0 + off:q0 + TCH],
                                            start=True, stop=True)
                                    ptp = p2.tile([128, 2 * TCH], F32R,
                                                  tag="pt", name=f"pt{x}")
                                    nc.scalar.activation(
                                        ptp[:, 0:TCH + w],
                                        stp[:, 0:TCH + w],
                                        AF.Exp, scale=0.125)
                                    if j >= 0:
                                        ptv = ptp[:].rearrange(
                                            "p (two t) -> p two t", two=2)
                                        nc.vector.tensor_tensor(
                                            ptv[:, :, 0:128], ptv[:, :, 0:128],
                                            mask2[:], OP.mult)
                                    ptps[(x, kt)] = ptp
                                if kt >= LAG:
                                    pk = kt - LAG
                                    off = offs.pop((x, pk))
                                    w = TCH - off
                                    ptp = ptps.pop((x, pk))
                                    for i in range(2):
                                        nc.tensor.matmul(
                                            po[hp][i][:, off:TCH],
                                            vt[:, pk, 2 * hp + i, :],
                                            ptp[:, i * TCH:i * TCH + w],
                                            start=(pk == 0),
                                            stop=(pk == nkt - 1))
                        # pass epilogue: unnormalized copy + denominator park
                        for x, hp in enumerate(hps):
                            for i in range(2):
                                prow = i * 64
                                nc.scalar.copy(
                                    attnT[prow:prow + 64, hp, q0:q0 + TCH],
                                    po[hp][i][0:D, :])
                                jrow = 32 * (2 * x + i)
                                nc.vector.tensor_copy(
                                    rcps[pidx][jrow:jrow + 1, :],
                                    po[hp][i][D:D + 1, :])
                        with nc.allow_low_precision(
                                reason="softmax denom recip"):
                            nc.vector.reciprocal(
                                rcps[pidx][:], rcps[pidx][:].bitcast(F32))

                # ---------------- phase 3: normalize + output projection -----
                with tc.tile_pool(name="p3o", bufs=3) as p3o, \
                     tc.tile_pool(name="ps3", bufs=2, space="PSUM") as ps3, \
                     tc.tile_pool(name="ps_bc", bufs=4, space="PSUM") as ps_bc:
                    for pidx, (ch, hpp) in enumerate(passes):
                        q0 = ch * TCH
                        for x, hp in enumerate((2 * hpp, 2 * hpp + 1)):
                            for i in range(2):
                                jrow = 32 * (2 * x + i)
                                bc = ps_bc.tile([64, TCH], F32, tag="bc",
                                                name="bc")
                                nc.tensor.matmul(
                                    bc[:], onesall_r[jrow:jrow + 1, :],
                                    rcps[pidx][jrow:jrow + 1, :],
                                    start=True, stop=True,
                                    tile_position=(jrow, 0))
                                prow = i * 64
                                nc.vector.tensor_tensor(
                                    attnT[prow:prow + 64, hp, q0:q0 + TCH],
                                    attnT[prow:prow + 64, hp, q0:q0 + TCH],
                                    bc[:], OP.mult)
                    for mc in range(8):
                        for tch in range(NCH):
                            pj = ps3.tile([128, TCH], F32, tag="pj")
                            for fc in range(4):
                                nc.tensor.matmul(
                                    pj[:], wp_t[:, fc, mc * 128:(mc + 1) * 128],
                                    attnT[:, fc, tch * TCH:(tch + 1) * TCH],
                                    start=(fc == 0), stop=(fc == 3))
                            ob = p3o.tile([128, TCH], F32, tag="ob")
                            nc.vector.tensor_copy(ob[:], pj[:])
                            nc.sync.dma_start(
                                outT_r[:, mc, tch * TCH:(tch + 1) * TCH], ob[:])
    return nc


# ---------------------------------------------------------------- cached runner
def _make_runner(with_bias=False):
    """Build nc once and return a callable(in_maps) -> list of out dicts,
    with the jitted sharded executable cached across calls."""
    import jax
    from jax.experimental.shard_map import shard_map
    from jax.sharding import Mesh, PartitionSpec

    import concourse.mybir as mybir
    from concourse import bass2jax

    nc = build_nc(with_bias=with_bias)

    partition_name = (nc.partition_id_tensor.name
                      if nc.partition_id_tensor else None)
    in_names = []
    out_names = []
    out_avals = []
    zero_shapes = []
    for alloc in nc.m.functions[0].allocations:
        if not isinstance(alloc, mybir.MemoryLocationSet):
            continue
        name = alloc.memorylocations[0].name
        if alloc.kind == "ExternalInput":
            if name != partition_name:
                in_names.append(name)
        elif alloc.kind == "ExternalOutput":
            out_names.append(name)
            shape = tuple(alloc.tensor_shape)
            dtype = mybir.dt.np(alloc.dtype)
            out_avals.append(jax.core.ShapedArray(shape, dtype))
            zero_shapes.append((shape, dtype))
    n_params = len(in_names)
    all_names = list(in_names + out_names)
    if partition_name is not None:
        all_names.append(partition_name)
    all_names = tuple(all_names)

    def _body(*args):
        operands = list(args)
        if partition_name is not None:
            operands.append(bass2jax.partition_id_tensor())
        outs = bass2jax._bass_exec_p.bind(
            *operands,
            out_avals=tuple(out_avals),
            in_names=all_names,
            out_names=tuple(out_names),
            lowering_input_output_aliases=(),
            sim_require_finite=True,
            sim_require_nnan=True,
            nc=nc,
        )
        return tuple(outs)

    devices = jax.devices()[:8]
    mesh = Mesh(np.asarray(devices), ("core",))
    n_outs = len(out_names)
    sharded = jax.jit(
        shard_map(
            _body, mesh=mesh,
            in_specs=(PartitionSpec("core"),) * (n_params + n_outs),
            out_specs=(PartitionSpec("core"),) * n_outs,
            check_rep=False,
        ),
        donate_argnums=tuple(range(n_params, n_params + n_outs)),
        keep_unused=True,
    )

    def run(in_maps):
        concat_in = [
            np.concatenate([np.asarray(m[name]) for m in in_maps], axis=0)
            for name in in_names
        ]
        concat_zeros = [
            np.zeros((8 * s[0], *s[1:]), dt) for s, dt in zero_shapes
        ]
        out_arrs = sharded(*concat_in, *concat_zeros)
        return [
            {
                name: np.asarray(out_arrs[i]).reshape(8, *zero_shapes[i][0])[c]
                for i, name in enumerate(out_names)
            }
            for c in range(8)
        ]

    return run


# ---------------------------------------------------------------- host wrapper
def make_in_maps(x, w_qkv, b_qkv, w_proj, b_proj):
    in_maps = []
    for b in range(B):
        xT_b = np.ascontiguousarray(x[b].T, dtype=np.float32)
        for hg in range(2):
            s = hg * 512
            wqk = np.ascontiguousarray(
                np.concatenate(
                    [w_qkv[:, s:s + 512], w_qkv[:, 1024 + s:1024 + s + 512]],
                    axis=1), dtype=np.float32)
            wv = np.ascontiguousarray(w_qkv[:, 2048 + s:2048 + s + 512],
                                      dtype=np.float32)
            wp = np.ascontiguousarray(w_proj[s:s + 512, :], dtype=np.float32)
            bqk = np.concatenate(
                [b_qkv[s:s + 512], b_qkv[1024 + s:1024 + s + 512]]
            ).reshape(1, 1024).astype(np.float32)
            bv = b_qkv[2048 + s:2048 + s + 512].reshape(1, 512).astype(np.float32)
            in_maps.append({"xT": xT_b, "w_qk": wqk, "w_v": wv, "w_p": wp,
                            "b_qk": bqk, "b_v": bv})
    return in_maps


def combine(results, b_proj):
    out = np.empty((B, T, C), dtype=np.float32)
    for b in range(B):
        acc = results[2 * b]["outT"] + results[2 * b + 1]["outT"]  # [1024, T]
        out[b] = acc.T
    out += b_proj.astype(np.float32)
    return out


def kernel(x, w_qkv, b_qkv, w_proj, b_proj):
    global _RUNNER
    b_qkv = np.asarray(b_qkv)
    with_bias = bool(np.any(b_qkv))
    if _RUNNER is None or _RUNNER[0] != with_bias:
        _RUNNER = (with_bias, _make_runner(with_bias=with_bias))
    in_maps = make_in_maps(np.asarray(x), np.asarray(w_qkv), b_qkv,
                           np.asarray(w_proj), np.asarray(b_proj))
    results = _RUNNER[1](in_maps)
    return combine(results, np.asarray(b_proj))


# revision 46
# speedup vs baseline: 1.8119x; 1.0988x over previous
"""Causal self-attention (B=4, T=2048, D=1024, H=16, d=64) on 8 TRN2 cores.

Sharding: 8 cores = 4 batches x 2 head-groups (8 heads each). Each core
computes, for its (batch, head-group):
  qk^T = (x @ w_qk)^T           [1024, T]   (q^T rows 0..511, k^T rows 512..1023)
  v    = x @ w_v                [T, 512]    (+ ones column per head -> [.., 65])
  S^T  = K^T.T @ Q^T per head   [k, q] tiles, exp via ACT (scale 1/8 folded)
  P^T causal-masked, AV: oT[65, q] accumulates V'.T @ P^T  (row 64 = denom)
  attn^T = oT[0:64] * (1/denom) (denominator broadcast via K=1 matmul)
  out^T += w_proj_slice.T-chunks @ attn^T  -> [1024, T] partial
Host sums the two head-group partials per batch, adds biases' linear terms,
and transposes back. All matmuls run as float32r (full-rate fp32).
"""

import json

import numpy as np

B = 4
T = 2048
C = 1024
NH = 8          # heads per core
D = 64
TCH = 512       # q/time chunk
NKT = T // 128  # 16 k-tiles
NCH = T // TCH  # 4 chunks

_RUNNER = None


# ---------------------------------------------------------------- BIR legalize
def _legalize_bir_json(bir_bytes):
    """Stock walrus allows only one sem wait per instruction; hoist extras
    onto same-engine NoOps inserted immediately before."""
    bir = json.loads(bir_bytes)
    n = [0]
    changed = False
    for func in bir.get("functions", []):
        for bb in func.get("blocks", []):
            out = []
            for inst in bb.get("instructions", []):
                si = inst.get("sync_info")
                if si:
                    upds = si.get("on_update") or []
                    assert len(upds) <= 1, (inst.get("name"), len(upds))
                waits = (si or {}).get("on_wait") or []
                if len(waits) > 1:
                    changed = True
                    for w in waits[:-1]:
                        n[0] += 1
                        out.append({
                            "debug": inst.get("debug", 0),
                            "engine": inst["engine"],
                            "ins": [],
                            "name": f"I-waitsplit-{n[0]}",
                            "opcode": "NoOp",
                            "outs": [],
                            "sync_info": {"on_update": [], "on_wait": [w]},
                        })
                    si["on_wait"] = [waits[-1]]
                out.append(inst)
            bb["instructions"] = out
    return json.dumps(bir).encode() if changed else bir_bytes


def _install_patch():
    import concourse.bass2jax as b2j
    import concourse.bass_utils as bu

    if getattr(bu, "_waitsplit_patched", False):
        return
    orig = bu.compile_bir_kernel

    def patched(bir_json, tmpdir, neff_name="file.neff"):
        if isinstance(bir_json, str):
            bir_json = bir_json.encode()
        return orig(_legalize_bir_json(bir_json), tmpdir, neff_name=neff_name)

    b2j.compile_bir_kernel = patched
    bu.compile_bir_kernel = patched
    bu._waitsplit_patched = True


# ---------------------------------------------------------------- bass program
def build_nc(with_bias=False):
    _install_patch()
    import concourse.bass as bass
    import concourse.mybir as mybir
    from concourse.tile import TileContext

    F32 = mybir.dt.float32
    F32R = mybir.dt.float32r
    AF = mybir.ActivationFunctionType
    OP = mybir.AluOpType

    nc = bass.Bass("TRN2")
    xT = nc.dram_tensor("xT", [C, T], F32R, kind="ExternalInput")
    w_qk = nc.dram_tensor("w_qk", [C, 1024], F32R, kind="ExternalInput")
    w_v = nc.dram_tensor("w_v", [C, 512], F32R, kind="ExternalInput")
    w_p = nc.dram_tensor("w_p", [512, 1024], F32R, kind="ExternalInput")
    b_qk = nc.dram_tensor("b_qk", [1, 1024], F32R, kind="ExternalInput")
    b_v = nc.dram_tensor("b_v", [1, 512], F32R, kind="ExternalInput")
    outT = nc.dram_tensor("outT", [C, T], F32, kind="ExternalOutput")

    xT_r = xT.rearrange("(co p) t -> p co t", p=128)      # [128, 8, T]
    wqk_r = w_qk.rearrange("(co p) n -> p co n", p=128)   # [128, 8, 1024]
    wv_r = w_v.rearrange("(co p) n -> p co n", p=128)     # [128, 8, 512]
    wp_r = w_p.rearrange("(fo p) n -> p fo n", p=128)     # [128, 4, 1024]
    outT_r = outT.rearrange("(mo p) t -> p mo t", p=128)  # [128, 8, T]

    with TileContext(nc) as tc:
        with tc.tile_pool(name="persist", bufs=1) as pp:
            qkT = pp.tile([128, 8, T], F32R)        # rows: q^T (0..3), k^T (4..7)
            # v' per (k-tile, head): [v (64) | ones (1)] -> AV lhsT [128, 65];
            # PSUM row 64 accumulates the softmax denominator.
            vt = pp.tile([128, NKT, NH, D + 1], F32R)
            onesp_f = pp.tile([128, 1], F32)
            nc.gpsimd.memset(onesp_f[:], 1.0)
            nc.vector.tensor_copy(
                vt[:, :, :, D:D + 1], onesp_f[:].to_broadcast([128, NKT, NH, 1]))
            onesbc_r = pp.tile([1, D], F32R)
            nc.vector.tensor_copy(onesbc_r[:],
                                  onesp_f[0:1, :].to_broadcast([1, D]))
            bqk_t = pp.tile([1, 1024], F32R)
            bv_t = pp.tile([1, 512], F32R)
            nc.sync.dma_start(bqk_t[:], b_qk[:])
            nc.sync.dma_start(bv_t[:], b_v[:])
            if with_bias:
                ones_f = pp.tile([1, TCH], F32)
                nc.gpsimd.memset(ones_f[:], 1.0)
                ones_r = pp.tile([1, TCH], F32R)
                nc.vector.tensor_copy(ones_r[:], ones_f[:])
                onesrow_r = pp.tile([1, 128], F32R)
                nc.vector.tensor_copy(
                    onesrow_r[:], onesp_f[0:1, :].to_broadcast([1, 128]))

            # ---------------- phase 1: qkv projections ----------------
            with tc.tile_pool(name="p1", bufs=1) as p1, \
                 tc.tile_pool(name="p1x", bufs=2) as p1x, \
                 tc.tile_pool(name="ps1", bufs=2, space="PSUM") as ps1, \
                 tc.tile_pool(name="ps1v", bufs=2, space="PSUM") as ps1v:
                wqk_t = p1.tile([128, 8, 1024], F32R)
                wv_t = p1.tile([128, 8, 512], F32R)
                nc.sync.dma_start(wqk_t[:], wqk_r[:])
                nc.sync.dma_start(wv_t[:], wv_r[:])
                for tch in range(NCH):
                    xt = p1x.tile([128, 8, TCH], F32R)
                    nc.sync.dma_start(xt[:], xT_r[:, :, tch * TCH:(tch + 1) * TCH])
                    for mc in range(8):
                        pq = ps1.tile([128, TCH], F32)
                        for cc in range(8):
                            nc.tensor.matmul(
                                pq[:], wqk_t[:, cc, mc * 128:(mc + 1) * 128],
                                xt[:, cc, :], start=(cc == 0),
                                stop=(cc == 7 and not with_bias))
                        if with_bias:
                            nc.tensor.matmul(
                                pq[:], bqk_t[:, mc * 128:(mc + 1) * 128],
                                ones_r[:], start=False, stop=True)
                        nc.vector.tensor_copy(
                            qkT[:, mc, tch * TCH:(tch + 1) * TCH], pq[:])
                    for tt in range(4):
                        pv = ps1v.tile([128, 512], F32)
                        for cc in range(8):
                            nc.tensor.matmul(
                                pv[:], xt[:, cc, tt * 128:(tt + 1) * 128],
                                wv_t[:, cc, :], start=(cc == 0),
                                stop=(cc == 7 and not with_bias))
                        if with_bias:
                            nc.tensor.matmul(
                                pv[:], onesrow_r[:], bv_t[:],
                                start=False, stop=True)
                        nc.vector.tensor_copy(
                            vt[:, tch * 4 + tt, :, 0:D],
                            pv[:].rearrange("p (h d) -> p h d", h=NH))

            # ---------------- phase 2: attention ----------------
            with tc.tile_pool(name="persist2", bufs=1) as pp2:
                attnT = pp2.tile([128, 4, T], F32R)
                # prefetch proj weights so phase 3 starts without a DMA stall
                wp_t = pp2.tile([128, 4, 1024], F32R)
                nc.sync.dma_start(wp_t[:], wp_r[:])
                # single 128x128 lower-tri mask, duplicated for the head pair
                mask2 = pp2.tile([128, 2, 128], F32)
                nc.gpsimd.memset(mask2[:], 1.0)
                for half in range(2):
                    nc.gpsimd.affine_select(
                        out=mask2[:, half, :], in_=mask2[:, half, :],
                        compare_op=OP.is_ge, fill=0.0,
                        base=0, pattern=[[1, 128]], channel_multiplier=-1)
                # per-pass denominator tiles: 4 heads' denominators parked at
                # partitions 0/32/64/96, one batched reciprocal per pass
                passes = [(ch, hpp) for ch in range(NCH) for hpp in range(2)]
                rcps = []
                for p in range(len(passes)):
                    r_ = pp2.tile([128, TCH], F32R, tag=f"rcp{p}",
                                  name=f"rcp{p}")
                    rcps.append(r_)
                onesall_r = pp2.tile([128, D], F32R)
                nc.vector.tensor_copy(onesall_r[:],
                                      onesp_f[:].to_broadcast([128, D]))

                LAG = 2
                with tc.tile_pool(name="p2", bufs=5) as p2, \
                     tc.tile_pool(name="ps_st", bufs=1, space="PSUM") as ps_st, \
                     tc.tile_pool(name="ps_ot", bufs=4, space="PSUM") as ps_ot:
                    for pidx, (ch, hpp) in enumerate(passes):
                        nkt = 4 * ch + 4   # k-tiles needed (causal)
                        q0 = ch * TCH
                        hps = (2 * hpp, 2 * hpp + 1)
                        # K=64 / M=65 matmuls never trigger the PE HAM
                        # warm-up; fire full 128x128 dummies so the whole
                        # pass runs at 2.4 GHz instead of 1.2 GHz.
                        n_warm = 10 if pidx == 0 else 2
                        warm = ps_st.tile([128, 2 * TCH], F32, tag="st0",
                                          name="warm")
                        for wi in range(n_warm):
                            nc.tensor.matmul(
                                warm[:, 0:TCH], wp_t[:, 0, 0:128],
                                qkT[:, 0, 0:TCH], start=True, stop=True)
                        po = {}
                        for hp in hps:
                            po[hp] = [ps_ot.tile([D + 1, TCH], F32, tag="ot",
                                                 name=f"po{hp}_{i}")
                                      for i in range(2)]
                        ptps = {}
                        offs = {}
                        for kt in range(nkt + LAG):
                            for x, hp in enumerate(hps):
                                if kt < nkt:
                                    j = kt - 4 * ch
                                    off = max(j, 0) * 128
                                    w = TCH - off
                                    offs[(x, kt)] = off
                                    stp = ps_st.tile([128, 2 * TCH], F32,
                                                     tag=f"st{x}",
                                                     name=f"st{x}")
                                    for i in range(2):
                                        prow = i * 64
                                        nc.tensor.matmul(
                                            stp[:, i * TCH:i * TCH + w],
                                            qkT[prow:prow + 64, 4 + hp,
                                                kt * 128:(kt + 1) * 128],
                                            qkT[prow:prow + 64, hp,
                                                q0 + off:q0 + TCH],
                                            start=True, stop=True)
                                    ptp = p2.tile([128, 2 * TCH], F32R,
                                                  tag="pt", name=f"pt{x}")
                                    nc.scalar.activation(
                                        ptp[:, 0:TCH + w],
                                        stp[:, 0:TCH + w],
                                        AF.Exp, scale=0.125)
                                    if j >= 0:
                                        ptv = ptp[:].rearrange(
                                            "p (two t) -> p two t", two=2)
                                        nc.vector.tensor_tensor(
                                            ptv[:, :, 0:128], ptv[:, :, 0:128],
                                            mask2[:], OP.mult)
                                    ptps[(x, kt)] = ptp
                                if kt >= LAG:
                                    pk = kt - LAG
                                    off = offs.pop((x, pk))
                                    w = TCH - off
                                    ptp = ptps.pop((x, pk))
                                    for i in range(2):
                                        nc.tensor.matmul(
                                            po[hp][i][:, off:TCH],
                                            vt[:, pk, 2 * hp + i, :],
                                            ptp[:, i * TCH:i * TCH + w],
                                            start=(pk == 0),
                                            stop=(pk == nkt - 1))
                        # pass epilogue: unnormalized copy + denominator park
                        for x, hp in enumerate(hps):
                            for i in range(2):
                                prow = i * 64
                                nc.vector.tensor_copy(
                                    attnT[prow:prow + 64, hp, q0:q0 + TCH],
                                    po[hp][i][0:D, :])
                                jrow = 32 * (2 * x + i)
                                nc.vector.tensor_copy(
                                    rcps[pidx][jrow:jrow + 1, :],
                                    po[hp][i][D:D + 1, :])
                        with nc.allow_low_precision(
                                reason="softmax denom recip"):
                            nc.vector.reciprocal(
                                rcps[pidx][:], rcps[pidx][:].bitcast(F32))

                # ---------------- phase 3: normalize + output projection -----
                with tc.tile_pool(name="p3o", bufs=3) as p3o, \
                     tc.tile_pool(name="ps3", bufs=2, space="PSUM") as ps3, \
                     tc.tile_pool(name="ps_bc", bufs=4, space="PSUM") as ps_bc:
                    for pidx, (ch, hpp) in enumerate(passes):
                        q0 = ch * TCH
                        for x, hp in enumerate((2 * hpp, 2 * hpp + 1)):
                            for i in range(2):
                                jrow = 32 * (2 * x + i)
                                bc = ps_bc.tile([64, TCH], F32, tag="bc",
                                                name="bc")
                                nc.tensor.matmul(
                                    bc[:], onesall_r[jrow:jrow + 1, :],
                                    rcps[pidx][jrow:jrow + 1, :],
                                    start=True, stop=True,
                                    tile_position=(jrow, 0))
                                prow = i * 64
                                nc.vector.tensor_tensor(
                                    attnT[prow:prow + 64, hp, q0:q0 + TCH],
                                    attnT[prow:prow + 64, hp, q0:q0 + TCH],
                                    bc[:], OP.mult)
                    for mc in range(8):
                        for tch in range(NCH):
                            pj = ps3.tile([128, TCH], F32, tag="pj")
                            for fc in range(4):
                                nc.tensor.matmul(
                                    pj[:], wp_t[:, fc, mc * 128:(mc + 1) * 128],
                                    attnT[:, fc, tch * TCH:(tch + 1) * TCH],
                                    start=(fc == 0), stop=(fc == 3))
                            ob = p3o.tile([128, TCH], F32, tag="ob")
                            nc.vector.tensor_copy(ob[:], pj[:])
                            nc.sync.dma_start(
                                outT_r[:, mc, tch * TCH:(tch + 1) * TCH], ob[:])
    return nc


# ---------------------------------------------------------------- cached runner
def _make_runner(with_bias=False):
    """Build nc once and return a callable(in_maps) -> list of out dicts,
    with the jitted sharded executable cached across calls."""
    import jax
    from jax.experimental.shard_map import shard_map
    from jax.sharding import Mesh, PartitionSpec

    import concourse.mybir as mybir
    from concourse import bass2jax

    nc = build_nc(with_bias=with_bias)

    partition_name = (nc.partition_id_tensor.name
                      if nc.partition_id_tensor else None)
    in_names = []
    out_names = []
    out_avals = []
    zero_shapes = []
    for alloc in nc.m.functions[0].allocations:
        if not isinstance(alloc, mybir.MemoryLocationSet):
            continue
        name = alloc.memorylocations[0].name
        if alloc.kind == "ExternalInput":
            if name != partition_name:
                in_names.append(name)
        elif alloc.kind == "ExternalOutput":
            out_names.append(name)
            shape = tuple(alloc.tensor_shape)
            dtype = mybir.dt.np(alloc.dtype)
            out_avals.append(jax.core.ShapedArray(shape, dtype))
            zero_shapes.append((shape, dtype))
    n_params = len(in_names)
    all_names = list(in_names + out_names)
    if partition_name is not None:
        all_names.append(partition_name)
    all_names = tuple(all_names)

    def _body(*args):
        operands = list(args)
        if partition_name is not None:
            operands.append(bass2jax.partition_id_tensor())
        outs = bass2jax._bass_exec_p.bind(
            *operands,
            out_avals=tuple(out_avals),
            in_names=all_names,
            out_names=tuple(out_names),
            lowering_input_output_aliases=(),
            sim_require_finite=True,
            sim_require_nnan=True,
            nc=nc,
        )
        return tuple(outs)

    devices = jax.devices()[:8]
    mesh = Mesh(np.asarray(devices), ("core",))
    n_outs = len(out_names)
    sharded = jax.jit(
        shard_map(
            _body, mesh=mesh,
            in_specs=(PartitionSpec("core"),) * (n_params + n_outs),
            out_specs=(PartitionSpec("core"),) * n_outs,
            check_rep=False,
        ),
        donate_argnums=tuple(range(n_params, n_params + n_outs)),
        keep_unused=True,
    )

    def run(in_maps):
        concat_in = [
            np.concatenate([np.asarray(m[name]) for m in in_maps], axis=0)
            for name in in_names
        ]
        concat_zeros = [
            np.zeros((8 * s[0], *s[1:]), dt) for s, dt in zero_shapes
        ]
        out_arrs = sharded(*concat_in, *concat_zeros)
        return [
            {
                name: np.asarray(out_arrs[i]).reshape(8, *zero_shapes[i][0])[c]
                for i, name in enumerate(out_names)
            }
            for c in range(8)
        ]

    return run


# ---------------------------------------------------------------- host wrapper
def make_in_maps(x, w_qkv, b_qkv, w_proj, b_proj):
    in_maps = []
    for b in range(B):
        xT_b = np.ascontiguousarray(x[b].T, dtype=np.float32)
        for hg in range(2):
            s = hg * 512
            wqk = np.ascontiguousarray(
                np.concatenate(
                    [w_qkv[:, s:s + 512], w_qkv[:, 1024 + s:1024 + s + 512]],
                    axis=1), dtype=np.float32)
            wv = np.ascontiguousarray(w_qkv[:, 2048 + s:2048 + s + 512],
                                      dtype=np.float32)
            wp = np.ascontiguousarray(w_proj[s:s + 512, :], dtype=np.float32)
            bqk = np.concatenate(
                [b_qkv[s:s + 512], b_qkv[1024 + s:1024 + s + 512]]
            ).reshape(1, 1024).astype(np.float32)
            bv = b_qkv[2048 + s:2048 + s + 512].reshape(1, 512).astype(np.float32)
            in_maps.append({"xT": xT_b, "w_qk": wqk, "w_v": wv, "w_p": wp,
                            "b_qk": bqk, "b_v": bv})
    return in_maps


def combine(results, b_proj):
    out = np.empty((B, T, C), dtype=np.float32)
    for b in range(B):
        acc = results[2 * b]["outT"] + results[2 * b + 1]["outT"]  # [1024, T]
        out[b] = acc.T
    out += b_proj.astype(np.float32)
    return out


def kernel(x, w_qkv, b_qkv, w_proj, b_proj):
    global _RUNNER
    b_qkv = np.asarray(b_qkv)
    with_bias = bool(np.any(b_qkv))
    if _RUNNER is None or _RUNNER[0] != with_bias:
        _RUNNER = (with_bias, _make_runner(with_bias=with_bias))
    in_maps = make_in_maps(np.asarray(x), np.asarray(w_qkv), b_qkv,
                           np.asarray(w_proj), np.asarray(b_proj))
    results = _RUNNER[1](in_maps)
    return combine(results, np.asarray(b_proj))
